# revision 21
# baseline (speedup 1.0000x reference)
"""Trainium2 Bass kernel for the BDH dense transformer (B=2, T=512, D=256, NH=4,
N=8192, 4 weight-tied layers, vocab 256).

Sharding: one (batch, head) pair per NeuronCore (2 x 4 = 8 cores). Per layer,
each core computes its head's yMLP partial (T, D); the 4 cores of a batch group
AllReduce the partials (with the replicated residual x/4 folded into the
payload) and every core redundantly applies layernorm so the activations stay
replicated within the group.

Collective overlap: phase D (y_latent/gate/yMLP) is split into two T-halves.
The first half's partial (t-chunks 0,1) is AllReduced while the second half
computes; the second collective is hidden under the NEXT layer's phase A first
T-half (which only needs the already-reduced t-chunks 0,1 of the new residual).

N-permutation: the score contraction over N is invariant under any permutation
of N applied consistently to (encoder cols, encoder_v cols, decoder rows, rope
freqs). We (1) de-interleave the rope pairs (even idx -> "lo", odd -> "hi") so
the pairwise rope becomes rotate-half form, then (2) interleave lo/hi CHUNK
QUADS: kernel chunks 8g..8g+7 = (lo 4g..4g+3, hi 4g..4g+3). A rope group is
then 8 consecutive chunks, relu/gate ops cover 2 adjacent chunks each, and
cos/sin tables are indexed by g.

Scores run in fp8e4 (DoubleRow, 2 N-chunks per matmul at 2x rate): the rope
scale C_ROPE is folded into the host cos/sin tables, qr tiles quantize to fp8
on the rope output, and the single 1/C^2 descale happens in the PSUM->smask
copy. All other matmuls are bf16 with fp32 PSUM accumulation.

All big DMAs are partition-major (host pre-transposes dec/cos/sin/emb/oneh/
lmh) so each of the 128 partition rows is one contiguous descriptor.
"""

import math

import numpy as np
import ml_dtypes

import concourse.bass as bass
import concourse.mybir as mybir
import concourse.tile as tile
from concourse import bacc
from concourse import bass_utils
from concourse.masks import make_identity

BF16 = ml_dtypes.bfloat16
F8E4 = ml_dtypes.float8_e4m3
F32 = mybir.dt.float32
BF = mybir.dt.bfloat16
F8 = mybir.dt.float8e4

# model dims (hardcoded per the problem spec)
B, T, D, NH, VOCAB = 2, 512, 256, 4, 256
N_LAYER = 4
MLP_MULT = 128
N = D * MLP_MULT // NH          # 8192 neurons per head
LN_EPS = 1e-5
TWO_PI = 2.0 * math.pi

NCH = N // 128                   # 64 partition-chunks of the neuron dim
HCH = NCH // 2                   # 32 chunks per rotate-half half
NGR = HCH // 4                   # 8 rope groups of 8 chunks (4 lo + 4 hi)
TC = T // 128                    # 4 t-chunks
DC = D // 128                    # 2 d-chunks
VC = VOCAB // 128                # 2 vocab-chunks
TH = T // 2                      # 256: columns per T-half

FP8_B = False                    # scores in fp8e4 DoubleRow (fp8 DVE writes too slow)
C_ENCV = 1024.0                  # fp8 scale on encoder_v (host-side)
C_YKV = 8.0                      # fp8 scale on the yKV cast (LN out can hit ~15-20)
C_ROPE = 32.0                    # rope-table scale folded into cos/sin
DESCALE = 1.0 / (C_ROPE * C_ROPE) if FP8_B else 1.0

RG = [[0, 1, 2, 3], [4, 5, 6, 7]]

AF = mybir.ActivationFunctionType
ALU = mybir.AluOpType
PM = mybir.MatmulPerfMode


def _layer_norm(nc, tmp, eps_tile, src, out_bf):
    """LN over the free dim (256 wide) of a (128, 256) tile -> bf16 out."""
    stats = tmp.tile([128, 6], F32, tag="bnst", bufs=2, name="ln_stats")
    nc.vector.bn_stats(stats, src)
    mv = tmp.tile([128, 2], F32, tag="bnmv", bufs=2, name="ln_mv")
    nc.vector.bn_aggr(mv, stats)
    std = tmp.tile([128, 1], F32, tag="std", bufs=2, name="ln_std")
    nc.scalar.activation(std, mv[:, 1:2], AF.Sqrt, bias=eps_tile)
    rstd = tmp.tile([128, 1], F32, tag="rstd", bufs=2, name="ln_rstd")
    nc.vector.reciprocal(rstd, std)
    nc.vector.tensor_scalar(
        out=out_bf, in0=src, scalar1=mv[:, 0:1], scalar2=rstd,
        op0=ALU.subtract, op1=ALU.mult,
    )


def _build_nc():
    nc = bacc.Bacc("TRN2", target_bir_lowering=False, debug=False, num_devices=8)

    # per-core external inputs (all partition-major)
    enc_w = nc.dram_tensor("enc_w", [DC, 128, N], BF, kind="ExternalInput")
    encv_w = nc.dram_tensor("encv_w", [DC, 128, N], BF, kind="ExternalInput")
    dec_w = nc.dram_tensor("dec_w", [128, NCH, D], BF, kind="ExternalInput")
    cos_w = nc.dram_tensor("cos_w", [128, NGR, 4, T], BF, kind="ExternalInput")
    sin_w = nc.dram_tensor("sin_w", [128, NGR, 4, T], BF, kind="ExternalInput")
    oneh_w = nc.dram_tensor("oneh_w", [128, VC, T], BF, kind="ExternalInput")
    emb_w = nc.dram_tensor("emb_w", [128, VC, D], BF, kind="ExternalInput")
    lmh_w = nc.dram_tensor("lmh_w", [128, DC, VOCAB], BF, kind="ExternalInput")
    logits_o = nc.dram_tensor("logits_o", [TC, 128, VOCAB], F32, kind="ExternalOutput")

    QRD = F8 if FP8_B else BF

    with tile.TileContext(nc) as tc:
        with (
            tc.tile_pool(name="wpool", bufs=1) as wp,
            tc.tile_pool(name="xspool", bufs=1) as xsp,
            tc.tile_pool(name="stream", bufs=3) as stream,
            tc.tile_pool(name="rope", bufs=2) as rope,
            tc.tile_pool(name="work", bufs=1) as work,
            tc.tile_pool(name="tmp", bufs=2) as tmp,
            tc.tile_pool(name="psA", bufs=2, space="PSUM") as psA,
            tc.tile_pool(name="psS", bufs=1, space="PSUM") as psS,
            tc.tile_pool(name="psT", bufs=1, space="PSUM") as psT,
            tc.tile_pool(name="dram", bufs=1, space="DRAM") as dram,
        ):
            # ---- resident weights (embedding inputs first so x0 can start
            # while the big encoder DMAs stream in) ----
            emb_sb = wp.tile([128, VC, D], BF, name="emb_sb")
            nc.sync.dma_start(emb_sb, emb_w.ap())
            oneh_sb = wp.tile([128, VC, T], BF, name="oneh_sb")
            nc.sync.dma_start(oneh_sb, oneh_w.ap())

            # warm up the collectives path with the same payload size as the
            # per-half AllReduce so layer 0 doesn't pay the first-call cost
            wcc_in = dram.tile([128, 2, D], BF, tag="wccin", name="wcc_in")
            wcc_out = dram.tile([128, 2, D], BF, tag="wccout", name="wcc_out")
            wcc_sb = tmp.tile([128, 2, D], BF, tag="wcc", bufs=1, name="wcc_sb")
            nc.vector.memset(wcc_sb, 0.0)
            nc.sync.dma_start(wcc_in, wcc_sb)
            nc.gpsimd.collective_compute(
                "AllReduce", ALU.add, replica_groups=RG,
                ins=[wcc_in.opt()], outs=[wcc_out.opt()],
            )

            ident = wp.tile([128, 128], BF, name="ident")
            make_identity(nc, ident)
            eps_tile = wp.tile([128, 1], F32, name="eps_tile")
            nc.vector.memset(eps_tile, LN_EPS)

            # encoder DMAs in slices (per-partition-contiguous columns)
            enc_sb = []
            for dc in range(DC):
                e = wp.tile([128, N], BF, tag=f"enc{dc}", name=f"enc_sb{dc}")
                enc_sb.append(e)
            NSL = 8
            sl = N // NSL
            for sli in range(NSL):
                for dc in range(DC):
                    nc.sync.dma_start(
                        enc_sb[dc][:, sli * sl:(sli + 1) * sl],
                        enc_w.ap()[dc, :, sli * sl:(sli + 1) * sl],
                    )
            encv_sb = []
            for dc in range(DC):
                ev = wp.tile([128, N], BF, tag=f"encv{dc}", name=f"encv_sb{dc}")
                encv_sb.append(ev)
            for sli in range(NSL):
                for dc in range(DC):
                    nc.sync.dma_start(
                        encv_sb[dc][:, sli * sl:(sli + 1) * sl],
                        encv_w.ap()[dc, :, sli * sl:(sli + 1) * sl],
                    )
            lmh_sb = wp.tile([128, DC, VOCAB], BF, name="lmh_sb")
            nc.sync.dma_start(lmh_sb, lmh_w.ap())

            # ---- persistent activations ----
            xs = xsp.tile([128, NCH, T], BF, name="xs")          # x_sparse / xy gate
            smask = work.tile([128, TC, T], BF, name="smask")    # masked scores (lhsT)
            x_bf = work.tile([128, TC, D], BF, name="x_bf")      # residual x (LN'd)
            xT = work.tile([128, DC, T], BF, name="xT")
            ykv_bf = work.tile([128, TC, D], BF, name="ykv_bf")
            ykvT = work.tile([128, 2, DC, TH], BF, name="ykvT")
            part_bf = work.tile([128, TC, D], BF, name="part_bf")

            # per-half collective buffers
            cc_in = [dram.tile([128, 2, D], BF, tag=f"ccin{h}", name=f"cc_in{h}")
                     for h in range(2)]
            cc_out = [dram.tile([128, 2, D], BF, tag=f"ccout{h}", name=f"cc_out{h}")
                      for h in range(2)]

            # zero the always-zero lower-left region of the masked score tiles
            for i in range(1, TC):
                nc.vector.memset(smask[:, i, 0:128 * i], 0.0)

            def emit_A_half(half):
                """x_latent columns [256h, 256h+256) for all chunks; relu in
                2-chunk pairs alternating scalar/vector. Chunk order follows
                the lo/hi pair interleave so rope group g is ready after
                chunks 4g..4g+3."""
                hs = slice(TH * half, TH * (half + 1))
                for pi in range(NCH // 2):
                    k0 = 2 * pi
                    # alternate between the psA lat buffers and the (idle
                    # during phase A) score banks so the relu round-trip
                    # never starves the tensor engine of PSUM space
                    if pi % 2 == 0:
                        lat = psA.tile([128, 2, TH], F32, tag="lat", bufs=3,
                                       name="lat_ps")
                    else:
                        lat = psS.tile([128, 2, TH], F32, tag=f"s{(pi // 2) % 4}",
                                       name="lat_ps")
                    for two in range(2):
                        k = k0 + two
                        for dc in range(DC):
                            nc.tensor.matmul(
                                lat[:, two, :],
                                lhsT=enc_sb[dc][:, 128 * k:128 * (k + 1)],
                                rhs=xT[:, dc, hs],
                                start=(dc == 0), stop=(dc == DC - 1),
                            )
                    if pi % 2 == 0:
                        nc.scalar.activation(xs[:, k0:k0 + 2, hs], lat, AF.Relu)
                    else:
                        nc.vector.tensor_scalar_max(xs[:, k0:k0 + 2, hs], lat, 0.0)

            def emit_tail(layer, half):
                """Consume the half's AllReduce result: LN -> x_bf, transpose
                into xT; on the last layer also the lm head + output DMA."""
                ag = tmp.tile([128, 2, D], BF, tag="ag", bufs=2, name="ag")
                nc.sync.dma_start(ag, cc_out[half])
                for jj in range(2):
                    j = 2 * half + jj
                    _layer_norm(nc, tmp, eps_tile, ag[:, jj, :], x_bf[:, j, :])
                    for dc in range(DC):
                        tp = psT.tile([128, 128], BF, tag="tr", name="tp_ps")
                        nc.tensor.transpose(
                            tp, x_bf[:, j, 128 * dc:128 * (dc + 1)], ident
                        )
                        nc.scalar.copy(xT[:, dc, 128 * j:128 * (j + 1)], tp)
                    if layer == N_LAYER - 1:
                        lg = psT.tile([128, VOCAB], F32, tag="tr", name="lg_ps")
                        for dc in range(DC):
                            nc.tensor.matmul(
                                lg, lhsT=xT[:, dc, 128 * j:128 * (j + 1)],
                                rhs=lmh_sb[:, dc, :],
                                start=(dc == 0), stop=(dc == DC - 1),
                            )
                        lgs = tmp.tile([128, VOCAB], F32, tag="lgs", name="lg_sb")
                        nc.scalar.copy(lgs, lg)
                        nc.sync.dma_start(logits_o.ap()[j], lgs)

            # ---- embedding: x0 = LN(onehot.T @ embed) ----
            for j in range(TC):
                x0 = psT.tile([128, D], F32, tag="tr", name="x0_ps")
                for vc in range(VC):
                    nc.tensor.matmul(
                        x0, lhsT=oneh_sb[:, vc, 128 * j:128 * (j + 1)],
                        rhs=emb_sb[:, vc, :],
                        start=(vc == 0), stop=(vc == VC - 1),
                    )
                _layer_norm(nc, tmp, eps_tile, x0, x_bf[:, j, :])
                for dc in range(DC):
                    tp = psT.tile([128, 128], BF, tag="tr", name="tp_ps")
                    nc.tensor.transpose(
                        tp, x_bf[:, j, 128 * dc:128 * (dc + 1)], ident
                    )
                    nc.scalar.copy(xT[:, dc, 128 * j:128 * (j + 1)], tp)
            emit_A_half(0)
            emit_A_half(1)

            # ---- layers ----
            for layer in range(N_LAYER):
                # phase B: rope (chunk groups of 4 = 2 lo-pairs + 2 hi-pairs)
                # + scores S = QR^T QR (upper-triangular blocks). With fp8,
                # each matmul is a DoubleRow pair (2 chunks, 2x rate).
                spsum = [
                    psS.tile([128, T], F32, tag=f"s{i}", name=f"score_ps{i}")
                    for i in range(TC)
                ]
                with nc.named_scope(f"L{layer}_B"):
                    for g in range(NGR):
                        lo = slice(8 * g, 8 * g + 4)
                        hi = slice(8 * g + 4, 8 * g + 8)
                        cos2 = rope.tile([128, 8, T], BF, tag="cosg", name="cos2")
                        nc.sync.dma_start(cos2[:, 0:4, :], cos_w.ap()[:, g])
                        nc.sync.dma_start(cos2[:, 4:8, :], cos_w.ap()[:, g])
                        sing = rope.tile([128, 4, T], BF, tag="sing", name="sing")
                        nc.sync.dma_start(sing, sin_w.ap()[:, g])
                        grp = slice(8 * g, 8 * g + 8)
                        lo = slice(8 * g, 8 * g + 4)
                        hi = slice(8 * g + 4, 8 * g + 8)
                        # one big multiply for both halves against cos, then
                        # qrlo/qrhi materialize in-place in the product tile
                        tc8 = rope.tile([128, 8, T], BF, tag="qrc", bufs=2,
                                        name="ropetc")
                        tb = rope.tile([128, 4, T], BF, tag="tb", bufs=2,
                                       name="ropetb")
                        tb2 = rope.tile([128, 4, T], BF, tag="tb", bufs=2,
                                        name="ropetb2")
                        nc.vector.tensor_mul(tc8, xs[:, grp, :], cos2)
                        nc.vector.tensor_mul(tb, xs[:, hi, :], sing)
                        nc.vector.tensor_sub(tc8[:, 0:4, :], tc8[:, 0:4, :], tb)
                        nc.vector.tensor_mul(tb2, xs[:, lo, :], sing)
                        nc.vector.tensor_add(tc8[:, 4:8, :], tc8[:, 4:8, :], tb2)
                        for q in range(8):
                            first = (g == 0) and (q == 0)
                            last = (g == NGR - 1) and (q == 7)
                            for i in range(TC):
                                nc.tensor.matmul(
                                    spsum[i][:, 128 * i:T],
                                    lhsT=tc8[:, q, 128 * i:128 * (i + 1)],
                                    rhs=tc8[:, q, 128 * i:T],
                                    start=first, stop=last,
                                )

                # phase C: descale+mask scores, attention out, LN, transpose
                with nc.named_scope(f"L{layer}_C"):
                    for i in range(TC):
                        src = spsum[i][:, 128 * i:T]
                        dst = smask[:, i, 128 * i:T]
                        if i % 2 == 0:
                            nc.vector.tensor_scalar_mul(dst, src, DESCALE)
                        else:
                            nc.scalar.activation(dst, src, AF.Copy, scale=DESCALE)
                        diag = smask[:, i, 128 * i:128 * (i + 1)]
                        nc.gpsimd.affine_select(
                            out=diag, in_=diag, pattern=[[1, 128]], base=0,
                            channel_multiplier=-1, compare_op=ALU.is_gt, fill=0.0,
                        )
                        att = psS.tile([128, D], F32, tag=f"s{i}", name="att_ps")
                        for ii in range(i + 1):
                            nc.tensor.matmul(
                                att, lhsT=smask[:, ii, 128 * i:128 * (i + 1)],
                                rhs=x_bf[:, ii, :],
                                start=(ii == 0), stop=(ii == i),
                            )
                        _layer_norm(nc, tmp, eps_tile, att, ykv_bf[:, i, :])
                        for dc in range(DC):
                            tp = psT.tile([128, 128], BF, tag="tr", name="tp_ps")
                            nc.tensor.transpose(
                                tp, ykv_bf[:, i, 128 * dc:128 * (dc + 1)], ident
                            )
                            nc.scalar.copy(
                                ykvT[:, i // 2, dc, 128 * (i % 2):128 * (i % 2 + 1)],
                                tp,
                            )


                # phase D (per T-half): y_latent, fused relu+gate (2-chunk
                # pairs), yMLP partials, then the half's AllReduce with the
                # residual x/4 folded into the payload. Software-pipelined:
                # mlp matmuls lag one pair behind ylat/gate.
                for half in range(2):
                    hs = slice(TH * half, TH * (half + 1))
                    js = (2 * half, 2 * half + 1)
                    with nc.named_scope(f"L{layer}_D{half}"):
                        mlp = {
                            j: psS.tile([128, D], F32, tag=f"s{j}", name=f"mlp_ps{j}")
                            for j in js
                        }
                        NP = NCH // 2
                        MLAG = 2
                        dec_pairs = {}
                        for p in range(NP + MLAG):
                            if p < NP:
                                k0 = 2 * p
                                dp = stream.tile([128, 2, D], BF, tag=f"dec{half}",
                                                 bufs=2, name="dec_t")
                                nc.sync.dma_start(dp, dec_w.ap()[:, k0:k0 + 2, :])
                                dec_pairs[p] = dp
                                ylat = psA.tile([128, 2, TH], F32, tag="lat",
                                                bufs=3, name="ylat_ps")
                                for two in range(2):
                                    k = k0 + two
                                    for dc in range(DC):
                                        nc.tensor.matmul(
                                            ylat[:, two, :],
                                            lhsT=encv_sb[dc][:, 128 * k:128 * (k + 1)],
                                            rhs=ykvT[:, half, dc, :],
                                            start=(dc == 0), stop=(dc == DC - 1),
                                        )
                                # xy_sparse = relu(ylat) * x_sparse, fused
                                nc.vector.scalar_tensor_tensor(
                                    out=xs[:, k0:k0 + 2, hs], in0=ylat, scalar=0.0,
                                    in1=xs[:, k0:k0 + 2, hs],
                                    op0=ALU.max, op1=ALU.mult,
                                )
                            pm = p - MLAG
                            if pm >= 0:
                                dpm = dec_pairs.pop(pm)
                                for two in range(2):
                                    km = 2 * pm + two
                                    for j in js:
                                        nc.tensor.matmul(
                                            mlp[j],
                                            lhsT=xs[:, km, 128 * j:128 * (j + 1)],
                                            rhs=dpm[:, two, :],
                                            start=(km == 0), stop=(km == NCH - 1),
                                        )
                        # payload = yMLP partial + x/4 (residual folded in)
                        for j in js:
                            nc.vector.scalar_tensor_tensor(
                                out=part_bf[:, j, :], in0=x_bf[:, j, :],
                                scalar=0.25, in1=mlp[j],
                                op0=ALU.mult, op1=ALU.add,
                            )
                        nc.sync.dma_start(cc_in[half], part_bf[:, 2 * half:2 * half + 2, :])
                        nc.gpsimd.collective_compute(
                            "AllReduce", ALU.add, replica_groups=RG,
                            ins=[cc_in[half].opt()], outs=[cc_out[half].opt()],
                        )

                # tails + next layer's phase A halves, interleaved so the
                # second collective hides under A's first T-half
                with nc.named_scope(f"L{layer}_E0"):
                    emit_tail(layer, 0)
                if layer < N_LAYER - 1:
                    with nc.named_scope(f"L{layer + 1}_A0"):
                        emit_A_half(0)
                with nc.named_scope(f"L{layer}_E1"):
                    emit_tail(layer, 1)
                if layer < N_LAYER - 1:
                    with nc.named_scope(f"L{layer + 1}_A1"):
                        emit_A_half(1)

    nc.compile()
    return nc


_NC_CACHE = None


def _get_nc():
    global _NC_CACHE
    if _NC_CACHE is None:
        _NC_CACHE = _build_nc()
    return _NC_CACHE


def _host_tables():
    # de-interleave rope pairs (even first), then interleave lo/hi chunk
    # pairs: kernel chunk 4g+{0,1} = lo chunks 2g,2g+1 (= even source idx),
    # kernel chunk 4g+{2,3} = hi chunks 2g,2g+1 (= odd source idx)
    deint = np.concatenate([np.arange(0, N, 2), np.arange(1, N, 2)])
    chunk_order = []
    for g in range(NGR):
        chunk_order += [4 * g + i for i in range(4)]
        chunk_order += [HCH + 4 * g + i for i in range(4)]
    perm = deint.reshape(NCH, 128)[chunk_order].reshape(N)

    tq = np.floor(np.arange(N, dtype=np.float64) / 2.0) * 2.0
    freqs = 1.0 / (2.0 ** 16) ** (tq / N) / TWO_PI
    phases = np.arange(T)[None, :] * freqs[:, None]      # (N, T)
    p = (phases % 1.0) * TWO_PI
    scale = C_ROPE if FP8_B else 1.0
    cos_full = (np.cos(p) * scale)[perm]                 # (N, T), kernel order
    sin_full = (np.sin(p) * scale)[perm]
    cosg = np.empty((128, NGR, 4, T), dtype=BF16)
    sing = np.empty((128, NGR, 4, T), dtype=BF16)
    for g in range(NGR):
        for two in range(4):
            k = 8 * g + two                              # lo chunk of group
            cosg[:, g, two, :] = cos_full[128 * k:128 * (k + 1), :].astype(BF16)
            sing[:, g, two, :] = sin_full[128 * k:128 * (k + 1), :].astype(BF16)
    return perm, cosg, sing


def make_in_maps(idx, embed, encoder, encoder_v, decoder, lm_head):
    perm, cos_t, sin_t = _host_tables()
    idx = np.asarray(idx)
    embed = np.asarray(embed, dtype=np.float32)
    enc = np.asarray(encoder, dtype=np.float32)[:, :, perm].astype(BF16)
    encv = np.asarray(encoder_v, dtype=np.float32)[:, :, perm].astype(BF16)
    dec = np.asarray(decoder, dtype=np.float32).reshape(NH, N, D)[:, perm, :].astype(BF16)
    # decoder partition-major: [128, NCH, D]
    dec_pm = np.ascontiguousarray(
        dec.reshape(NH, NCH, 128, D).transpose(0, 2, 1, 3))
    emb_pm = np.ascontiguousarray(
        embed.astype(BF16).reshape(VC, 128, D).transpose(1, 0, 2))
    lmh_pm = np.ascontiguousarray(
        np.asarray(lm_head, dtype=np.float32).astype(BF16)
        .reshape(DC, 128, VOCAB).transpose(1, 0, 2))

    oneh = np.zeros((B, VOCAB, T), dtype=BF16)           # (b, v, t) = onehot^T
    for b in range(B):
        oneh[b, np.asarray(idx[b], dtype=np.int64), np.arange(T)] = 1
    oneh_pm = np.ascontiguousarray(
        oneh.reshape(B, VC, 128, T).transpose(0, 2, 1, 3))

    in_maps = []
    for c in range(8):
        b, h = c // 4, c % 4
        in_maps.append({
            "enc_w": np.ascontiguousarray(enc[h].reshape(DC, 128, N)),
            "encv_w": np.ascontiguousarray(encv[h].reshape(DC, 128, N)),
            "dec_w": dec_pm[h],
            "cos_w": cos_t,
            "sin_w": sin_t,
            "oneh_w": oneh_pm[b],
            "emb_w": emb_pm,
            "lmh_w": lmh_pm,
        })
    return in_maps


def kernel(idx, embed, encoder, encoder_v, decoder, lm_head):
    nc = _get_nc()
    in_maps = make_in_maps(idx, embed, encoder, encoder_v, decoder, lm_head)
    res = bass_utils.run_bass_kernel_spmd(nc, in_maps, core_ids=list(range(8)))
    out = np.empty((B, T, VOCAB), dtype=np.float32)
    for b in range(B):
        out[b] = res.results[4 * b]["logits_o"].reshape(T, VOCAB)
    return out


# revision 22
# speedup vs baseline: 1.0697x; 1.0697x over previous
"""Trainium2 Bass kernel for the BDH dense transformer (B=2, T=512, D=256, NH=4,
N=8192, 4 weight-tied layers, vocab 256).

Sharding: one (batch, head) pair per NeuronCore (2 x 4 = 8 cores). Per layer,
each core computes its head's yMLP partial (T, D); the 4 cores of a batch group
AllReduce the partials (with the replicated residual x/4 folded into the
payload) and every core redundantly applies layernorm so the activations stay
replicated within the group.

Collective overlap: phase D (y_latent/gate/yMLP) is split into two T-halves.
The first half's partial (t-chunks 0,1) is AllReduced while the second half
computes; the second collective is hidden under the NEXT layer's phase A first
T-half (which only needs the already-reduced t-chunks 0,1 of the new residual).

N-permutation: the score contraction over N is invariant under any permutation
of N applied consistently to (encoder cols, encoder_v cols, decoder rows, rope
freqs). We (1) de-interleave the rope pairs (even idx -> "lo", odd -> "hi") so
the pairwise rope becomes rotate-half form, then (2) interleave lo/hi CHUNK
QUADS: kernel chunks 8g..8g+7 = (lo 4g..4g+3, hi 4g..4g+3). A rope group is
then 8 consecutive chunks, relu/gate ops cover 2 adjacent chunks each, and
cos/sin tables are indexed by g.

Scores run in fp8e4 (DoubleRow, 2 N-chunks per matmul at 2x rate): the rope
scale C_ROPE is folded into the host cos/sin tables, qr tiles quantize to fp8
on the rope output, and the single 1/C^2 descale happens in the PSUM->smask
copy. All other matmuls are bf16 with fp32 PSUM accumulation.

All big DMAs are partition-major (host pre-transposes dec/cos/sin/emb/oneh/
lmh) so each of the 128 partition rows is one contiguous descriptor.
"""

import math

import numpy as np
import ml_dtypes

import concourse.bass as bass
import concourse.mybir as mybir
import concourse.tile as tile
from concourse import bacc
from concourse import bass_utils
from concourse.masks import make_identity

BF16 = ml_dtypes.bfloat16
F8E4 = ml_dtypes.float8_e4m3
F32 = mybir.dt.float32
BF = mybir.dt.bfloat16
F8 = mybir.dt.float8e4

# model dims (hardcoded per the problem spec)
B, T, D, NH, VOCAB = 2, 512, 256, 4, 256
N_LAYER = 4
MLP_MULT = 128
N = D * MLP_MULT // NH          # 8192 neurons per head
LN_EPS = 1e-5
TWO_PI = 2.0 * math.pi

NCH = N // 128                   # 64 partition-chunks of the neuron dim
HCH = NCH // 2                   # 32 chunks per rotate-half half
NGR = HCH // 4                   # 8 rope groups of 8 chunks (4 lo + 4 hi)
TC = T // 128                    # 4 t-chunks
DC = D // 128                    # 2 d-chunks
VC = VOCAB // 128                # 2 vocab-chunks
TH = T // 2                      # 256: columns per T-half

FP8_B = False                    # scores in fp8e4 DoubleRow (fp8 DVE writes too slow)
C_ENCV = 1024.0                  # fp8 scale on encoder_v (host-side)
C_YKV = 8.0                      # fp8 scale on the yKV cast (LN out can hit ~15-20)
C_ROPE = 32.0                    # rope-table scale folded into cos/sin
DESCALE = 1.0 / (C_ROPE * C_ROPE) if FP8_B else 1.0

RG = [[0, 1, 2, 3], [4, 5, 6, 7]]

AF = mybir.ActivationFunctionType
ALU = mybir.AluOpType
PM = mybir.MatmulPerfMode


def _layer_norm(nc, tmp, eps_tile, src, out_bf):
    """LN over the free dim (256 wide) of a (128, 256) tile -> bf16 out."""
    stats = tmp.tile([128, 6], F32, tag="bnst", bufs=2, name="ln_stats")
    nc.vector.bn_stats(stats, src)
    mv = tmp.tile([128, 2], F32, tag="bnmv", bufs=2, name="ln_mv")
    nc.vector.bn_aggr(mv, stats)
    std = tmp.tile([128, 1], F32, tag="std", bufs=2, name="ln_std")
    nc.scalar.activation(std, mv[:, 1:2], AF.Sqrt, bias=eps_tile)
    rstd = tmp.tile([128, 1], F32, tag="rstd", bufs=2, name="ln_rstd")
    nc.vector.reciprocal(rstd, std)
    nc.vector.tensor_scalar(
        out=out_bf, in0=src, scalar1=mv[:, 0:1], scalar2=rstd,
        op0=ALU.subtract, op1=ALU.mult,
    )


def _build_nc():
    nc = bacc.Bacc("TRN2", target_bir_lowering=False, debug=False, num_devices=8)

    # per-core external inputs (all partition-major)
    enc_w = nc.dram_tensor("enc_w", [DC, 128, N], BF, kind="ExternalInput")
    encv_w = nc.dram_tensor("encv_w", [DC, 128, N], BF, kind="ExternalInput")
    dec_w = nc.dram_tensor("dec_w", [128, NCH, D], BF, kind="ExternalInput")
    cos_w = nc.dram_tensor("cos_w", [128, NGR, 4, T], BF, kind="ExternalInput")
    sin_w = nc.dram_tensor("sin_w", [128, NGR, 4, T], BF, kind="ExternalInput")
    oneh_w = nc.dram_tensor("oneh_w", [128, VC, T], BF, kind="ExternalInput")
    emb_w = nc.dram_tensor("emb_w", [128, VC, D], BF, kind="ExternalInput")
    lmh_w = nc.dram_tensor("lmh_w", [128, DC, VOCAB], BF, kind="ExternalInput")
    logits_o = nc.dram_tensor("logits_o", [TC, 128, VOCAB], F32, kind="ExternalOutput")

    QRD = F8 if FP8_B else BF

    with tile.TileContext(nc) as tc:
        with (
            tc.tile_pool(name="wpool", bufs=1) as wp,
            tc.tile_pool(name="xspool", bufs=1) as xsp,
            tc.tile_pool(name="stream", bufs=3) as stream,
            tc.tile_pool(name="rope", bufs=2) as rope,
            tc.tile_pool(name="work", bufs=1) as work,
            tc.tile_pool(name="tmp", bufs=2) as tmp,
            tc.tile_pool(name="psA", bufs=2, space="PSUM") as psA,
            tc.tile_pool(name="psS", bufs=1, space="PSUM") as psS,
            tc.tile_pool(name="psT", bufs=1, space="PSUM") as psT,
            tc.tile_pool(name="dram", bufs=1, space="DRAM") as dram,
        ):
            # ---- resident weights (embedding inputs first so x0 can start
            # while the big encoder DMAs stream in) ----
            emb_sb = wp.tile([128, VC, D], BF, name="emb_sb")
            nc.sync.dma_start(emb_sb, emb_w.ap())
            oneh_sb = wp.tile([128, VC, T], BF, name="oneh_sb")
            nc.sync.dma_start(oneh_sb, oneh_w.ap())

            # warm up the collectives path with the same payload size as the
            # per-half AllReduce so layer 0 doesn't pay the first-call cost
            wcc_in = dram.tile([128, 2, D], BF, tag="wccin", name="wcc_in")
            wcc_out = dram.tile([128, 2, D], BF, tag="wccout", name="wcc_out")
            wcc_sb = tmp.tile([128, 2, D], BF, tag="wcc", bufs=1, name="wcc_sb")
            nc.vector.memset(wcc_sb, 0.0)
            nc.sync.dma_start(wcc_in, wcc_sb)
            nc.gpsimd.collective_compute(
                "AllReduce", ALU.add, replica_groups=RG,
                ins=[wcc_in.opt()], outs=[wcc_out.opt()],
            )

            ident = wp.tile([128, 128], BF, name="ident")
            make_identity(nc, ident)
            eps_tile = wp.tile([128, 1], F32, name="eps_tile")
            nc.vector.memset(eps_tile, LN_EPS)

            # encoder DMAs in slices (per-partition-contiguous columns)
            enc_sb = []
            for dc in range(DC):
                e = wp.tile([128, N], BF, tag=f"enc{dc}", name=f"enc_sb{dc}")
                enc_sb.append(e)
            NSL = 8
            sl = N // NSL
            for sli in range(NSL):
                for dc in range(DC):
                    nc.sync.dma_start(
                        enc_sb[dc][:, sli * sl:(sli + 1) * sl],
                        enc_w.ap()[dc, :, sli * sl:(sli + 1) * sl],
                    )
            encv_sb = []
            for dc in range(DC):
                ev = wp.tile([128, N], BF, tag=f"encv{dc}", name=f"encv_sb{dc}")
                encv_sb.append(ev)
            for sli in range(NSL):
                for dc in range(DC):
                    nc.sync.dma_start(
                        encv_sb[dc][:, sli * sl:(sli + 1) * sl],
                        encv_w.ap()[dc, :, sli * sl:(sli + 1) * sl],
                    )
            lmh_sb = wp.tile([128, DC, VOCAB], BF, name="lmh_sb")
            nc.sync.dma_start(lmh_sb, lmh_w.ap())

            # ---- persistent activations ----
            xs = xsp.tile([128, NCH, T], BF, name="xs")          # x_sparse / xy gate
            smask = work.tile([128, TC, T], BF, name="smask")    # masked scores (lhsT)
            x_bf = work.tile([128, TC, D], BF, name="x_bf")      # residual x (LN'd)
            xT = work.tile([128, DC, T], BF, name="xT")
            ykv_bf = work.tile([128, TC, D], BF, name="ykv_bf")
            ykvT = work.tile([128, 2, DC, TH], BF, name="ykvT")
            part_bf = work.tile([128, TC, D], BF, name="part_bf")

            # per-half collective buffers
            cc_in = [dram.tile([128, 2, D], BF, tag=f"ccin{h}", name=f"cc_in{h}")
                     for h in range(2)]
            cc_out = [dram.tile([128, 2, D], BF, tag=f"ccout{h}", name=f"cc_out{h}")
                      for h in range(2)]

            # zero the always-zero lower-left region of the masked score tiles
            for i in range(1, TC):
                nc.vector.memset(smask[:, i, 0:128 * i], 0.0)

            def emit_A_half(half):
                """x_latent columns [256h, 256h+256) for all chunks; relu in
                2-chunk pairs alternating scalar/vector. Chunk order follows
                the lo/hi pair interleave so rope group g is ready after
                chunks 4g..4g+3."""
                hs = slice(TH * half, TH * (half + 1))
                for pi in range(NCH // 2):
                    k0 = 2 * pi
                    # alternate between the psA lat buffers and the (idle
                    # during phase A) score banks so the relu round-trip
                    # never starves the tensor engine of PSUM space
                    if pi % 2 == 0:
                        lat = psA.tile([128, 2, TH], F32, tag="lat", bufs=3,
                                       name="lat_ps")
                    else:
                        lat = psS.tile([128, 2, TH], F32, tag=f"s{(pi // 2) % 4}",
                                       name="lat_ps")
                    for two in range(2):
                        k = k0 + two
                        for dc in range(DC):
                            nc.tensor.matmul(
                                lat[:, two, :],
                                lhsT=enc_sb[dc][:, 128 * k:128 * (k + 1)],
                                rhs=xT[:, dc, hs],
                                start=(dc == 0), stop=(dc == DC - 1),
                            )
                    if pi % 2 == 0:
                        nc.scalar.activation(xs[:, k0:k0 + 2, hs], lat, AF.Relu)
                    else:
                        nc.vector.tensor_scalar_max(xs[:, k0:k0 + 2, hs], lat, 0.0)

            def emit_tail(layer, half):
                """Consume the half's AllReduce result: LN -> x_bf, transpose
                into xT; on the last layer also the lm head + output DMA."""
                ag = tmp.tile([128, 2, D], BF, tag="ag", bufs=2, name="ag")
                nc.sync.dma_start(ag, cc_out[half])
                for jj in range(2):
                    j = 2 * half + jj
                    _layer_norm(nc, tmp, eps_tile, ag[:, jj, :], x_bf[:, j, :])
                    for dc in range(DC):
                        tp = psT.tile([128, 128], BF, tag="tr", name="tp_ps")
                        nc.tensor.transpose(
                            tp, x_bf[:, j, 128 * dc:128 * (dc + 1)], ident
                        )
                        nc.scalar.copy(xT[:, dc, 128 * j:128 * (j + 1)], tp)
                    if layer == N_LAYER - 1:
                        lg = psT.tile([128, VOCAB], F32, tag="tr", name="lg_ps")
                        for dc in range(DC):
                            nc.tensor.matmul(
                                lg, lhsT=xT[:, dc, 128 * j:128 * (j + 1)],
                                rhs=lmh_sb[:, dc, :],
                                start=(dc == 0), stop=(dc == DC - 1),
                            )
                        lgs = tmp.tile([128, VOCAB], F32, tag="lgs", name="lg_sb")
                        nc.scalar.copy(lgs, lg)
                        nc.sync.dma_start(logits_o.ap()[j], lgs)

            # ---- embedding: x0 = LN(onehot.T @ embed) ----
            for j in range(TC):
                x0 = psT.tile([128, D], F32, tag="tr", name="x0_ps")
                for vc in range(VC):
                    nc.tensor.matmul(
                        x0, lhsT=oneh_sb[:, vc, 128 * j:128 * (j + 1)],
                        rhs=emb_sb[:, vc, :],
                        start=(vc == 0), stop=(vc == VC - 1),
                    )
                _layer_norm(nc, tmp, eps_tile, x0, x_bf[:, j, :])
                for dc in range(DC):
                    tp = psT.tile([128, 128], BF, tag="tr", name="tp_ps")
                    nc.tensor.transpose(
                        tp, x_bf[:, j, 128 * dc:128 * (dc + 1)], ident
                    )
                    nc.scalar.copy(xT[:, dc, 128 * j:128 * (j + 1)], tp)
            emit_A_half(0)
            emit_A_half(1)

            # ---- layers ----
            for layer in range(N_LAYER):
                # phase B: rope (chunk groups of 4 = 2 lo-pairs + 2 hi-pairs)
                # + scores S = QR^T QR (upper-triangular blocks). With fp8,
                # each matmul is a DoubleRow pair (2 chunks, 2x rate).
                spsum = [
                    psS.tile([128, T], F32, tag=f"s{i}", name=f"score_ps{i}")
                    for i in range(TC)
                ]
                with nc.named_scope(f"L{layer}_B"):
                    for g in range(NGR):
                        lo = slice(8 * g, 8 * g + 4)
                        hi = slice(8 * g + 4, 8 * g + 8)
                        cosg = rope.tile([128, 4, T], BF, tag="cosg", name="cosg")
                        nc.sync.dma_start(cosg, cos_w.ap()[:, g])
                        sing = rope.tile([128, 4, T], BF, tag="sing", name="sing")
                        nc.sync.dma_start(sing, sin_w.ap()[:, g])
                        qrlo = rope.tile([128, 4, T], QRD, tag="qrlo", name="qrlo")
                        qrhi = rope.tile([128, 4, T], QRD, tag="qrhi", name="qrhi")
                        lo = slice(8 * g, 8 * g + 4)
                        hi = slice(8 * g + 4, 8 * g + 8)
                        ta = rope.tile([128, 4, T], BF, tag="ta", bufs=1, name="ropeta")
                        tb = rope.tile([128, 4, T], BF, tag="tb", bufs=1, name="ropetb")
                        nc.vector.tensor_mul(ta, xs[:, lo, :], cosg)
                        nc.vector.tensor_mul(tb, xs[:, hi, :], sing)
                        nc.vector.tensor_sub(qrlo, ta, tb)
                        ta2 = rope.tile([128, 4, T], BF, tag="ta", bufs=1, name="ropeta2")
                        tb2 = rope.tile([128, 4, T], BF, tag="tb", bufs=1, name="ropetb2")
                        nc.vector.tensor_mul(ta2, xs[:, hi, :], cosg)
                        nc.vector.tensor_mul(tb2, xs[:, lo, :], sing)
                        nc.vector.tensor_add(qrhi, ta2, tb2)
                        for qi, qr in enumerate((qrlo, qrhi)):
                            for kk in range(4):
                                first = (g == 0) and (qi == 0) and (kk == 0)
                                last = (g == NGR - 1) and (qi == 1) and (kk == 3)
                                for i in range(TC):
                                    nc.tensor.matmul(
                                        spsum[i][:, 128 * i:T],
                                        lhsT=qr[:, kk, 128 * i:128 * (i + 1)],
                                        rhs=qr[:, kk, 128 * i:T],
                                        start=first, stop=last,
                                    )

                # phase C: descale+mask scores, attention out, LN, transpose
                with nc.named_scope(f"L{layer}_C"):
                    for i in range(TC):
                        src = spsum[i][:, 128 * i:T]
                        dst = smask[:, i, 128 * i:T]
                        if i % 2 == 0:
                            nc.vector.tensor_scalar_mul(dst, src, DESCALE)
                        else:
                            nc.scalar.activation(dst, src, AF.Copy, scale=DESCALE)
                        diag = smask[:, i, 128 * i:128 * (i + 1)]
                        nc.gpsimd.affine_select(
                            out=diag, in_=diag, pattern=[[1, 128]], base=0,
                            channel_multiplier=-1, compare_op=ALU.is_gt, fill=0.0,
                        )
                        att = psS.tile([128, D], F32, tag=f"s{i}", name="att_ps")
                        for ii in range(i + 1):
                            nc.tensor.matmul(
                                att, lhsT=smask[:, ii, 128 * i:128 * (i + 1)],
                                rhs=x_bf[:, ii, :],
                                start=(ii == 0), stop=(ii == i),
                            )
                        _layer_norm(nc, tmp, eps_tile, att, ykv_bf[:, i, :])
                        for dc in range(DC):
                            tp = psT.tile([128, 128], BF, tag="tr", name="tp_ps")
                            nc.tensor.transpose(
                                tp, ykv_bf[:, i, 128 * dc:128 * (dc + 1)], ident
                            )
                            nc.scalar.copy(
                                ykvT[:, i // 2, dc, 128 * (i % 2):128 * (i % 2 + 1)],
                                tp,
                            )


                # phase D (per T-half): y_latent, fused relu+gate (2-chunk
                # pairs), yMLP partials, then the half's AllReduce with the
                # residual x/4 folded into the payload. Software-pipelined:
                # mlp matmuls lag one pair behind ylat/gate.
                for half in range(2):
                    hs = slice(TH * half, TH * (half + 1))
                    js = (2 * half, 2 * half + 1)
                    with nc.named_scope(f"L{layer}_D{half}"):
                        mlp = {
                            j: psS.tile([128, D], F32, tag=f"s{j}", name=f"mlp_ps{j}")
                            for j in js
                        }
                        NP = NCH // 2
                        MLAG = 2
                        dec_pairs = {}
                        for p in range(NP + MLAG):
                            if p < NP:
                                k0 = 2 * p
                                dp = stream.tile([128, 2, D], BF, tag=f"dec{half}",
                                                 bufs=4, name="dec_t")
                                nc.sync.dma_start(dp, dec_w.ap()[:, k0:k0 + 2, :])
                                dec_pairs[p] = dp
                                ylat = psA.tile([128, 2, TH], F32, tag="lat",
                                                bufs=3, name="ylat_ps")
                                for two in range(2):
                                    k = k0 + two
                                    for dc in range(DC):
                                        nc.tensor.matmul(
                                            ylat[:, two, :],
                                            lhsT=encv_sb[dc][:, 128 * k:128 * (k + 1)],
                                            rhs=ykvT[:, half, dc, :],
                                            start=(dc == 0), stop=(dc == DC - 1),
                                        )
                                # xy_sparse = relu(ylat) * x_sparse, fused
                                nc.vector.scalar_tensor_tensor(
                                    out=xs[:, k0:k0 + 2, hs], in0=ylat, scalar=0.0,
                                    in1=xs[:, k0:k0 + 2, hs],
                                    op0=ALU.max, op1=ALU.mult,
                                )
                            pm = p - MLAG
                            if pm >= 0:
                                dpm = dec_pairs.pop(pm)
                                for two in range(2):
                                    km = 2 * pm + two
                                    for j in js:
                                        nc.tensor.matmul(
                                            mlp[j],
                                            lhsT=xs[:, km, 128 * j:128 * (j + 1)],
                                            rhs=dpm[:, two, :],
                                            start=(km == 0), stop=(km == NCH - 1),
                                        )
                        # payload = yMLP partial + x/4 (residual folded in)
                        for j in js:
                            nc.vector.scalar_tensor_tensor(
                                out=part_bf[:, j, :], in0=x_bf[:, j, :],
                                scalar=0.25, in1=mlp[j],
                                op0=ALU.mult, op1=ALU.add,
                            )
                        nc.sync.dma_start(cc_in[half], part_bf[:, 2 * half:2 * half + 2, :])
                        nc.gpsimd.collective_compute(
                            "AllReduce", ALU.add, replica_groups=RG,
                            ins=[cc_in[half].opt()], outs=[cc_out[half].opt()],
                        )

                # tails + next layer's phase A halves, interleaved so the
                # second collective hides under A's first T-half
                with nc.named_scope(f"L{layer}_E0"):
                    emit_tail(layer, 0)
                if layer < N_LAYER - 1:
                    with nc.named_scope(f"L{layer + 1}_A0"):
                        emit_A_half(0)
                with nc.named_scope(f"L{layer}_E1"):
                    emit_tail(layer, 1)
                if layer < N_LAYER - 1:
                    with nc.named_scope(f"L{layer + 1}_A1"):
                        emit_A_half(1)

    nc.compile()
    return nc


_NC_CACHE = None


def _get_nc():
    global _NC_CACHE
    if _NC_CACHE is None:
        _NC_CACHE = _build_nc()
    return _NC_CACHE


def _host_tables():
    # de-interleave rope pairs (even first), then interleave lo/hi chunk
    # pairs: kernel chunk 4g+{0,1} = lo chunks 2g,2g+1 (= even source idx),
    # kernel chunk 4g+{2,3} = hi chunks 2g,2g+1 (= odd source idx)
    deint = np.concatenate([np.arange(0, N, 2), np.arange(1, N, 2)])
    chunk_order = []
    for g in range(NGR):
        chunk_order += [4 * g + i for i in range(4)]
        chunk_order += [HCH + 4 * g + i for i in range(4)]
    perm = deint.reshape(NCH, 128)[chunk_order].reshape(N)

    tq = np.floor(np.arange(N, dtype=np.float64) / 2.0) * 2.0
    freqs = 1.0 / (2.0 ** 16) ** (tq / N) / TWO_PI
    phases = np.arange(T)[None, :] * freqs[:, None]      # (N, T)
    p = (phases % 1.0) * TWO_PI
    scale = C_ROPE if FP8_B else 1.0
    cos_full = (np.cos(p) * scale)[perm]                 # (N, T), kernel order
    sin_full = (np.sin(p) * scale)[perm]
    cosg = np.empty((128, NGR, 4, T), dtype=BF16)
    sing = np.empty((128, NGR, 4, T), dtype=BF16)
    for g in range(NGR):
        for two in range(4):
            k = 8 * g + two                              # lo chunk of group
            cosg[:, g, two, :] = cos_full[128 * k:128 * (k + 1), :].astype(BF16)
            sing[:, g, two, :] = sin_full[128 * k:128 * (k + 1), :].astype(BF16)
    return perm, cosg, sing


def make_in_maps(idx, embed, encoder, encoder_v, decoder, lm_head):
    perm, cos_t, sin_t = _host_tables()
    idx = np.asarray(idx)
    embed = np.asarray(embed, dtype=np.float32)
    enc = np.asarray(encoder, dtype=np.float32)[:, :, perm].astype(BF16)
    encv = np.asarray(encoder_v, dtype=np.float32)[:, :, perm].astype(BF16)
    dec = np.asarray(decoder, dtype=np.float32).reshape(NH, N, D)[:, perm, :].astype(BF16)
    # decoder partition-major: [128, NCH, D]
    dec_pm = np.ascontiguousarray(
        dec.reshape(NH, NCH, 128, D).transpose(0, 2, 1, 3))
    emb_pm = np.ascontiguousarray(
        embed.astype(BF16).reshape(VC, 128, D).transpose(1, 0, 2))
    lmh_pm = np.ascontiguousarray(
        np.asarray(lm_head, dtype=np.float32).astype(BF16)
        .reshape(DC, 128, VOCAB).transpose(1, 0, 2))

    oneh = np.zeros((B, VOCAB, T), dtype=BF16)           # (b, v, t) = onehot^T
    for b in range(B):
        oneh[b, np.asarray(idx[b], dtype=np.int64), np.arange(T)] = 1
    oneh_pm = np.ascontiguousarray(
        oneh.reshape(B, VC, 128, T).transpose(0, 2, 1, 3))

    in_maps = []
    for c in range(8):
        b, h = c // 4, c % 4
        in_maps.append({
            "enc_w": np.ascontiguousarray(enc[h].reshape(DC, 128, N)),
            "encv_w": np.ascontiguousarray(encv[h].reshape(DC, 128, N)),
            "dec_w": dec_pm[h],
            "cos_w": cos_t,
            "sin_w": sin_t,
            "oneh_w": oneh_pm[b],
            "emb_w": emb_pm,
            "lmh_w": lmh_pm,
        })
    return in_maps


def kernel(idx, embed, encoder, encoder_v, decoder, lm_head):
    nc = _get_nc()
    in_maps = make_in_maps(idx, embed, encoder, encoder_v, decoder, lm_head)
    res = bass_utils.run_bass_kernel_spmd(nc, in_maps, core_ids=list(range(8)))
    out = np.empty((B, T, VOCAB), dtype=np.float32)
    for b in range(B):
        out[b] = res.results[4 * b]["logits_o"].reshape(T, VOCAB)
    return out


# revision 23
# speedup vs baseline: 1.0758x; 1.0057x over previous
"""Trainium2 Bass kernel for the BDH dense transformer (B=2, T=512, D=256, NH=4,
N=8192, 4 weight-tied layers, vocab 256).

Sharding: one (batch, head) pair per NeuronCore (2 x 4 = 8 cores). Per layer,
each core computes its head's yMLP partial (T, D); the 4 cores of a batch group
AllReduce the partials (with the replicated residual x/4 folded into the
payload) and every core redundantly applies layernorm so the activations stay
replicated within the group.

Collective overlap: phase D (y_latent/gate/yMLP) is split into two T-halves.
The first half's partial (t-chunks 0,1) is AllReduced while the second half
computes; the second collective is hidden under the NEXT layer's phase A first
T-half (which only needs the already-reduced t-chunks 0,1 of the new residual).

N-permutation: the score contraction over N is invariant under any permutation
of N applied consistently to (encoder cols, encoder_v cols, decoder rows, rope
freqs). We (1) de-interleave the rope pairs (even idx -> "lo", odd -> "hi") so
the pairwise rope becomes rotate-half form, then (2) interleave lo/hi CHUNK
QUADS: kernel chunks 8g..8g+7 = (lo 4g..4g+3, hi 4g..4g+3). A rope group is
then 8 consecutive chunks, relu/gate ops cover 2 adjacent chunks each, and
cos/sin tables are indexed by g.

Scores run in fp8e4 (DoubleRow, 2 N-chunks per matmul at 2x rate): the rope
scale C_ROPE is folded into the host cos/sin tables, qr tiles quantize to fp8
on the rope output, and the single 1/C^2 descale happens in the PSUM->smask
copy. All other matmuls are bf16 with fp32 PSUM accumulation.

All big DMAs are partition-major (host pre-transposes dec/cos/sin/emb/oneh/
lmh) so each of the 128 partition rows is one contiguous descriptor.
"""

import math

import numpy as np
import ml_dtypes

import concourse.bass as bass
import concourse.mybir as mybir
import concourse.tile as tile
from concourse import bacc
from concourse import bass_utils
from concourse.masks import make_identity

BF16 = ml_dtypes.bfloat16
F8E4 = ml_dtypes.float8_e4m3
F32 = mybir.dt.float32
BF = mybir.dt.bfloat16
F8 = mybir.dt.float8e4

# model dims (hardcoded per the problem spec)
B, T, D, NH, VOCAB = 2, 512, 256, 4, 256
N_LAYER = 4
MLP_MULT = 128
N = D * MLP_MULT // NH          # 8192 neurons per head
LN_EPS = 1e-5
TWO_PI = 2.0 * math.pi

NCH = N // 128                   # 64 partition-chunks of the neuron dim
HCH = NCH // 2                   # 32 chunks per rotate-half half
NGR = HCH // 4                   # 8 rope groups of 8 chunks (4 lo + 4 hi)
TC = T // 128                    # 4 t-chunks
DC = D // 128                    # 2 d-chunks
VC = VOCAB // 128                # 2 vocab-chunks
TH = T // 2                      # 256: columns per T-half

FP8_B = False                    # scores in fp8e4 DoubleRow (fp8 DVE writes too slow)
C_ENCV = 1024.0                  # fp8 scale on encoder_v (host-side)
C_YKV = 8.0                      # fp8 scale on the yKV cast (LN out can hit ~15-20)
C_ROPE = 32.0                    # rope-table scale folded into cos/sin
DESCALE = 1.0 / (C_ROPE * C_ROPE) if FP8_B else 1.0

RG = [[0, 1, 2, 3], [4, 5, 6, 7]]

AF = mybir.ActivationFunctionType
ALU = mybir.AluOpType
PM = mybir.MatmulPerfMode


def _layer_norm(nc, tmp, eps_tile, src, out_bf):
    """LN over the free dim (256 wide) of a (128, 256) tile -> bf16 out."""
    stats = tmp.tile([128, 6], F32, tag="bnst", bufs=2, name="ln_stats")
    nc.vector.bn_stats(stats, src)
    mv = tmp.tile([128, 2], F32, tag="bnmv", bufs=2, name="ln_mv")
    nc.vector.bn_aggr(mv, stats)
    std = tmp.tile([128, 1], F32, tag="std", bufs=2, name="ln_std")
    nc.scalar.activation(std, mv[:, 1:2], AF.Sqrt, bias=eps_tile)
    rstd = tmp.tile([128, 1], F32, tag="rstd", bufs=2, name="ln_rstd")
    nc.vector.reciprocal(rstd, std)
    nc.vector.tensor_scalar(
        out=out_bf, in0=src, scalar1=mv[:, 0:1], scalar2=rstd,
        op0=ALU.subtract, op1=ALU.mult,
    )


def _build_nc():
    nc = bacc.Bacc("TRN2", target_bir_lowering=False, debug=False, num_devices=8)

    # per-core external inputs (all partition-major)
    enc_w = nc.dram_tensor("enc_w", [DC, 128, N], BF, kind="ExternalInput")
    encv_w = nc.dram_tensor("encv_w", [DC, 128, N], BF, kind="ExternalInput")
    dec_w = nc.dram_tensor("dec_w", [128, NCH, D], BF, kind="ExternalInput")
    cos_w = nc.dram_tensor("cos_w", [128, NGR, 4, T], BF, kind="ExternalInput")
    sin_w = nc.dram_tensor("sin_w", [128, NGR, 4, T], BF, kind="ExternalInput")
    oneh_w = nc.dram_tensor("oneh_w", [128, VC, T], BF, kind="ExternalInput")
    emb_w = nc.dram_tensor("emb_w", [128, VC, D], BF, kind="ExternalInput")
    lmh_w = nc.dram_tensor("lmh_w", [128, DC, VOCAB], BF, kind="ExternalInput")
    logits_o = nc.dram_tensor("logits_o", [TC, 128, VOCAB], F32, kind="ExternalOutput")

    QRD = F8 if FP8_B else BF

    with tile.TileContext(nc) as tc:
        with (
            tc.tile_pool(name="wpool", bufs=1) as wp,
            tc.tile_pool(name="xspool", bufs=1) as xsp,
            tc.tile_pool(name="stream", bufs=3) as stream,
            tc.tile_pool(name="rope", bufs=2) as rope,
            tc.tile_pool(name="work", bufs=1) as work,
            tc.tile_pool(name="tmp", bufs=2) as tmp,
            tc.tile_pool(name="psA", bufs=2, space="PSUM") as psA,
            tc.tile_pool(name="psS", bufs=1, space="PSUM") as psS,
            tc.tile_pool(name="psT", bufs=1, space="PSUM") as psT,
            tc.tile_pool(name="dram", bufs=1, space="DRAM") as dram,
        ):
            # ---- resident weights (embedding inputs first so x0 can start
            # while the big encoder DMAs stream in) ----
            emb_sb = wp.tile([128, VC, D], BF, name="emb_sb")
            nc.sync.dma_start(emb_sb, emb_w.ap())
            oneh_sb = wp.tile([128, VC, T], BF, name="oneh_sb")
            nc.sync.dma_start(oneh_sb, oneh_w.ap())

            # warm up the collectives path with the same payload size as the
            # per-half AllReduce so layer 0 doesn't pay the first-call cost
            wcc_in = dram.tile([128, 2, D], BF, tag="wccin", name="wcc_in")
            wcc_out = dram.tile([128, 2, D], BF, tag="wccout", name="wcc_out")
            wcc_sb = tmp.tile([128, 2, D], BF, tag="wcc", bufs=1, name="wcc_sb")
            nc.vector.memset(wcc_sb, 0.0)
            nc.sync.dma_start(wcc_in, wcc_sb)
            nc.gpsimd.collective_compute(
                "AllReduce", ALU.add, replica_groups=RG,
                ins=[wcc_in.opt()], outs=[wcc_out.opt()],
            )

            ident = wp.tile([128, 128], BF, name="ident")
            make_identity(nc, ident)
            eps_tile = wp.tile([128, 1], F32, name="eps_tile")
            nc.vector.memset(eps_tile, LN_EPS)

            # encoder DMAs in slices (per-partition-contiguous columns)
            enc_sb = []
            for dc in range(DC):
                e = wp.tile([128, N], BF, tag=f"enc{dc}", name=f"enc_sb{dc}")
                enc_sb.append(e)
            NSL = 8
            sl = N // NSL
            for sli in range(NSL):
                for dc in range(DC):
                    nc.sync.dma_start(
                        enc_sb[dc][:, sli * sl:(sli + 1) * sl],
                        enc_w.ap()[dc, :, sli * sl:(sli + 1) * sl],
                    )
            encv_sb = []
            for dc in range(DC):
                ev = wp.tile([128, N], BF, tag=f"encv{dc}", name=f"encv_sb{dc}")
                encv_sb.append(ev)
            for sli in range(NSL):
                for dc in range(DC):
                    nc.sync.dma_start(
                        encv_sb[dc][:, sli * sl:(sli + 1) * sl],
                        encv_w.ap()[dc, :, sli * sl:(sli + 1) * sl],
                    )
            lmh_sb = wp.tile([128, DC, VOCAB], BF, name="lmh_sb")
            nc.sync.dma_start(lmh_sb, lmh_w.ap())

            # ---- persistent activations ----
            xs = xsp.tile([128, NCH, T], BF, name="xs")          # x_sparse / xy gate
            smask = work.tile([128, TC, T], BF, name="smask")    # masked scores (lhsT)
            x_bf = work.tile([128, TC, D], BF, name="x_bf")      # residual x (LN'd)
            xT = work.tile([128, DC, T], BF, name="xT")
            ykv_bf = work.tile([128, TC, D], BF, name="ykv_bf")
            ykvT = work.tile([128, 2, DC, TH], BF, name="ykvT")
            part_bf = work.tile([128, TC, D], BF, name="part_bf")

            # per-half collective buffers
            cc_in = [dram.tile([128, 2, D], BF, tag=f"ccin{h}", name=f"cc_in{h}")
                     for h in range(2)]
            cc_out = [dram.tile([128, 2, D], BF, tag=f"ccout{h}", name=f"cc_out{h}")
                      for h in range(2)]

            # zero the always-zero lower-left region of the masked score tiles
            for i in range(1, TC):
                nc.vector.memset(smask[:, i, 0:128 * i], 0.0)

            def emit_A_half(half):
                """x_latent columns [256h, 256h+256) for all chunks; relu in
                2-chunk pairs alternating scalar/vector. Chunk order follows
                the lo/hi pair interleave so rope group g is ready after
                chunks 4g..4g+3."""
                hs = slice(TH * half, TH * (half + 1))
                for pi in range(NCH // 2):
                    k0 = 2 * pi
                    # alternate between the psA lat buffers and the (idle
                    # during phase A) score banks so the relu round-trip
                    # never starves the tensor engine of PSUM space
                    if pi % 2 == 0:
                        lat = psA.tile([128, 2, TH], F32, tag="lat", bufs=3,
                                       name="lat_ps")
                    else:
                        lat = psS.tile([128, 2, TH], F32, tag=f"s{(pi // 2) % 4}",
                                       name="lat_ps")
                    for two in range(2):
                        k = k0 + two
                        for dc in range(DC):
                            nc.tensor.matmul(
                                lat[:, two, :],
                                lhsT=enc_sb[dc][:, 128 * k:128 * (k + 1)],
                                rhs=xT[:, dc, hs],
                                start=(dc == 0), stop=(dc == DC - 1),
                            )
                    # 2:1 scalar-biased relu split: keeps A tensor-paced
                    # while freeing vector time for the rope that paces B
                    if pi % 3 != 2:
                        nc.scalar.activation(xs[:, k0:k0 + 2, hs], lat, AF.Relu)
                    else:
                        nc.vector.tensor_scalar_max(xs[:, k0:k0 + 2, hs], lat, 0.0)

            def emit_tail(layer, half):
                """Consume the half's AllReduce result: LN -> x_bf, transpose
                into xT; on the last layer also the lm head + output DMA."""
                ag = tmp.tile([128, 2, D], BF, tag="ag", bufs=2, name="ag")
                nc.sync.dma_start(ag, cc_out[half])
                for jj in range(2):
                    j = 2 * half + jj
                    _layer_norm(nc, tmp, eps_tile, ag[:, jj, :], x_bf[:, j, :])
                    for dc in range(DC):
                        tp = psT.tile([128, 128], BF, tag="tr", name="tp_ps")
                        nc.tensor.transpose(
                            tp, x_bf[:, j, 128 * dc:128 * (dc + 1)], ident
                        )
                        nc.scalar.copy(xT[:, dc, 128 * j:128 * (j + 1)], tp)
                    if layer == N_LAYER - 1:
                        lg = psT.tile([128, VOCAB], F32, tag="tr", name="lg_ps")
                        for dc in range(DC):
                            nc.tensor.matmul(
                                lg, lhsT=xT[:, dc, 128 * j:128 * (j + 1)],
                                rhs=lmh_sb[:, dc, :],
                                start=(dc == 0), stop=(dc == DC - 1),
                            )
                        lgs = tmp.tile([128, VOCAB], F32, tag="lgs", name="lg_sb")
                        nc.scalar.copy(lgs, lg)
                        nc.sync.dma_start(logits_o.ap()[j], lgs)

            # ---- embedding: x0 = LN(onehot.T @ embed) ----
            for j in range(TC):
                x0 = psT.tile([128, D], F32, tag="tr", name="x0_ps")
                for vc in range(VC):
                    nc.tensor.matmul(
                        x0, lhsT=oneh_sb[:, vc, 128 * j:128 * (j + 1)],
                        rhs=emb_sb[:, vc, :],
                        start=(vc == 0), stop=(vc == VC - 1),
                    )
                _layer_norm(nc, tmp, eps_tile, x0, x_bf[:, j, :])
                for dc in range(DC):
                    tp = psT.tile([128, 128], BF, tag="tr", name="tp_ps")
                    nc.tensor.transpose(
                        tp, x_bf[:, j, 128 * dc:128 * (dc + 1)], ident
                    )
                    nc.scalar.copy(xT[:, dc, 128 * j:128 * (j + 1)], tp)
            emit_A_half(0)
            emit_A_half(1)

            # ---- layers ----
            for layer in range(N_LAYER):
                # phase B: rope (chunk groups of 4 = 2 lo-pairs + 2 hi-pairs)
                # + scores S = QR^T QR (upper-triangular blocks). With fp8,
                # each matmul is a DoubleRow pair (2 chunks, 2x rate).
                spsum = [
                    psS.tile([128, T], F32, tag=f"s{i}", name=f"score_ps{i}")
                    for i in range(TC)
                ]
                with nc.named_scope(f"L{layer}_B"):
                    for g in range(NGR):
                        lo = slice(8 * g, 8 * g + 4)
                        hi = slice(8 * g + 4, 8 * g + 8)
                        cosg = rope.tile([128, 4, T], BF, tag="cosg", name="cosg")
                        nc.sync.dma_start(cosg, cos_w.ap()[:, g])
                        sing = rope.tile([128, 4, T], BF, tag="sing", name="sing")
                        nc.sync.dma_start(sing, sin_w.ap()[:, g])
                        qrlo = rope.tile([128, 4, T], QRD, tag="qrlo", name="qrlo")
                        qrhi = rope.tile([128, 4, T], QRD, tag="qrhi", name="qrhi")
                        lo = slice(8 * g, 8 * g + 4)
                        hi = slice(8 * g + 4, 8 * g + 8)
                        ta = rope.tile([128, 4, T], BF, tag="ta", bufs=1, name="ropeta")
                        tb = rope.tile([128, 4, T], BF, tag="tb", bufs=1, name="ropetb")
                        nc.vector.tensor_mul(ta, xs[:, lo, :], cosg)
                        nc.vector.tensor_mul(tb, xs[:, hi, :], sing)
                        nc.vector.tensor_sub(qrlo, ta, tb)
                        ta2 = rope.tile([128, 4, T], BF, tag="ta", bufs=1, name="ropeta2")
                        tb2 = rope.tile([128, 4, T], BF, tag="tb", bufs=1, name="ropetb2")
                        nc.vector.tensor_mul(ta2, xs[:, hi, :], cosg)
                        nc.vector.tensor_mul(tb2, xs[:, lo, :], sing)
                        nc.vector.tensor_add(qrhi, ta2, tb2)
                        for qi, qr in enumerate((qrlo, qrhi)):
                            for kk in range(4):
                                first = (g == 0) and (qi == 0) and (kk == 0)
                                last = (g == NGR - 1) and (qi == 1) and (kk == 3)
                                for i in range(TC):
                                    nc.tensor.matmul(
                                        spsum[i][:, 128 * i:T],
                                        lhsT=qr[:, kk, 128 * i:128 * (i + 1)],
                                        rhs=qr[:, kk, 128 * i:T],
                                        start=first, stop=last,
                                    )

                # phase C: descale+mask scores, attention out, LN, transpose
                with nc.named_scope(f"L{layer}_C"):
                    for i in range(TC):
                        src = spsum[i][:, 128 * i:T]
                        dst = smask[:, i, 128 * i:T]
                        if i % 2 == 0:
                            nc.vector.tensor_scalar_mul(dst, src, DESCALE)
                        else:
                            nc.scalar.activation(dst, src, AF.Copy, scale=DESCALE)
                        diag = smask[:, i, 128 * i:128 * (i + 1)]
                        nc.gpsimd.affine_select(
                            out=diag, in_=diag, pattern=[[1, 128]], base=0,
                            channel_multiplier=-1, compare_op=ALU.is_gt, fill=0.0,
                        )
                        att = psS.tile([128, D], F32, tag=f"s{i}", name="att_ps")
                        for ii in range(i + 1):
                            nc.tensor.matmul(
                                att, lhsT=smask[:, ii, 128 * i:128 * (i + 1)],
                                rhs=x_bf[:, ii, :],
                                start=(ii == 0), stop=(ii == i),
                            )
                        _layer_norm(nc, tmp, eps_tile, att, ykv_bf[:, i, :])
                        for dc in range(DC):
                            tp = psT.tile([128, 128], BF, tag="tr", name="tp_ps")
                            nc.tensor.transpose(
                                tp, ykv_bf[:, i, 128 * dc:128 * (dc + 1)], ident
                            )
                            nc.scalar.copy(
                                ykvT[:, i // 2, dc, 128 * (i % 2):128 * (i % 2 + 1)],
                                tp,
                            )


                # phase D (per T-half): y_latent, fused relu+gate (2-chunk
                # pairs), yMLP partials, then the half's AllReduce with the
                # residual x/4 folded into the payload. Software-pipelined:
                # mlp matmuls lag one pair behind ylat/gate.
                for half in range(2):
                    hs = slice(TH * half, TH * (half + 1))
                    js = (2 * half, 2 * half + 1)
                    with nc.named_scope(f"L{layer}_D{half}"):
                        mlp = {
                            j: psS.tile([128, D], F32, tag=f"s{j}", name=f"mlp_ps{j}")
                            for j in js
                        }
                        NP = NCH // 2
                        MLAG = 2
                        dec_pairs = {}
                        for p in range(NP + MLAG):
                            if p < NP:
                                k0 = 2 * p
                                dp = stream.tile([128, 2, D], BF, tag=f"dec{half}",
                                                 bufs=4, name="dec_t")
                                nc.sync.dma_start(dp, dec_w.ap()[:, k0:k0 + 2, :])
                                dec_pairs[p] = dp
                                ylat = psA.tile([128, 2, TH], F32, tag="lat",
                                                bufs=3, name="ylat_ps")
                                for two in range(2):
                                    k = k0 + two
                                    for dc in range(DC):
                                        nc.tensor.matmul(
                                            ylat[:, two, :],
                                            lhsT=encv_sb[dc][:, 128 * k:128 * (k + 1)],
                                            rhs=ykvT[:, half, dc, :],
                                            start=(dc == 0), stop=(dc == DC - 1),
                                        )
                                # xy_sparse = relu(ylat) * x_sparse, fused
                                nc.vector.scalar_tensor_tensor(
                                    out=xs[:, k0:k0 + 2, hs], in0=ylat, scalar=0.0,
                                    in1=xs[:, k0:k0 + 2, hs],
                                    op0=ALU.max, op1=ALU.mult,
                                )
                            pm = p - MLAG
                            if pm >= 0:
                                dpm = dec_pairs.pop(pm)
                                for two in range(2):
                                    km = 2 * pm + two
                                    for j in js:
                                        nc.tensor.matmul(
                                            mlp[j],
                                            lhsT=xs[:, km, 128 * j:128 * (j + 1)],
                                            rhs=dpm[:, two, :],
                                            start=(km == 0), stop=(km == NCH - 1),
                                        )
                        # payload = yMLP partial + x/4 (residual folded in)
                        for j in js:
                            nc.vector.scalar_tensor_tensor(
                                out=part_bf[:, j, :], in0=x_bf[:, j, :],
                                scalar=0.25, in1=mlp[j],
                                op0=ALU.mult, op1=ALU.add,
                            )
                        nc.sync.dma_start(cc_in[half], part_bf[:, 2 * half:2 * half + 2, :])
                        nc.gpsimd.collective_compute(
                            "AllReduce", ALU.add, replica_groups=RG,
                            ins=[cc_in[half].opt()], outs=[cc_out[half].opt()],
                        )

                # tails + next layer's phase A halves, interleaved so the
                # second collective hides under A's first T-half
                with nc.named_scope(f"L{layer}_E0"):
                    emit_tail(layer, 0)
                if layer < N_LAYER - 1:
                    with nc.named_scope(f"L{layer + 1}_A0"):
                        emit_A_half(0)
                with nc.named_scope(f"L{layer}_E1"):
                    emit_tail(layer, 1)
                if layer < N_LAYER - 1:
                    with nc.named_scope(f"L{layer + 1}_A1"):
                        emit_A_half(1)

    nc.compile()
    return nc


_NC_CACHE = None


def _get_nc():
    global _NC_CACHE
    if _NC_CACHE is None:
        _NC_CACHE = _build_nc()
    return _NC_CACHE


def _host_tables():
    # de-interleave rope pairs (even first), then interleave lo/hi chunk
    # pairs: kernel chunk 4g+{0,1} = lo chunks 2g,2g+1 (= even source idx),
    # kernel chunk 4g+{2,3} = hi chunks 2g,2g+1 (= odd source idx)
    deint = np.concatenate([np.arange(0, N, 2), np.arange(1, N, 2)])
    chunk_order = []
    for g in range(NGR):
        chunk_order += [4 * g + i for i in range(4)]
        chunk_order += [HCH + 4 * g + i for i in range(4)]
    perm = deint.reshape(NCH, 128)[chunk_order].reshape(N)

    tq = np.floor(np.arange(N, dtype=np.float64) / 2.0) * 2.0
    freqs = 1.0 / (2.0 ** 16) ** (tq / N) / TWO_PI
    phases = np.arange(T)[None, :] * freqs[:, None]      # (N, T)
    p = (phases % 1.0) * TWO_PI
    scale = C_ROPE if FP8_B else 1.0
    cos_full = (np.cos(p) * scale)[perm]                 # (N, T), kernel order
    sin_full = (np.sin(p) * scale)[perm]
    cosg = np.empty((128, NGR, 4, T), dtype=BF16)
    sing = np.empty((128, NGR, 4, T), dtype=BF16)
    for g in range(NGR):
        for two in range(4):
            k = 8 * g + two                              # lo chunk of group
            cosg[:, g, two, :] = cos_full[128 * k:128 * (k + 1), :].astype(BF16)
            sing[:, g, two, :] = sin_full[128 * k:128 * (k + 1), :].astype(BF16)
    return perm, cosg, sing


def make_in_maps(idx, embed, encoder, encoder_v, decoder, lm_head):
    perm, cos_t, sin_t = _host_tables()
    idx = np.asarray(idx)
    embed = np.asarray(embed, dtype=np.float32)
    enc = np.asarray(encoder, dtype=np.float32)[:, :, perm].astype(BF16)
    encv = np.asarray(encoder_v, dtype=np.float32)[:, :, perm].astype(BF16)
    dec = np.asarray(decoder, dtype=np.float32).reshape(NH, N, D)[:, perm, :].astype(BF16)
    # decoder partition-major: [128, NCH, D]
    dec_pm = np.ascontiguousarray(
        dec.reshape(NH, NCH, 128, D).transpose(0, 2, 1, 3))
    emb_pm = np.ascontiguousarray(
        embed.astype(BF16).reshape(VC, 128, D).transpose(1, 0, 2))
    lmh_pm = np.ascontiguousarray(
        np.asarray(lm_head, dtype=np.float32).astype(BF16)
        .reshape(DC, 128, VOCAB).transpose(1, 0, 2))

    oneh = np.zeros((B, VOCAB, T), dtype=BF16)           # (b, v, t) = onehot^T
    for b in range(B):
        oneh[b, np.asarray(idx[b], dtype=np.int64), np.arange(T)] = 1
    oneh_pm = np.ascontiguousarray(
        oneh.reshape(B, VC, 128, T).transpose(0, 2, 1, 3))

    in_maps = []
    for c in range(8):
        b, h = c // 4, c % 4
        in_maps.append({
            "enc_w": np.ascontiguousarray(enc[h].reshape(DC, 128, N)),
            "encv_w": np.ascontiguousarray(encv[h].reshape(DC, 128, N)),
            "dec_w": dec_pm[h],
            "cos_w": cos_t,
            "sin_w": sin_t,
            "oneh_w": oneh_pm[b],
            "emb_w": emb_pm,
            "lmh_w": lmh_pm,
        })
    return in_maps


def kernel(idx, embed, encoder, encoder_v, decoder, lm_head):
    nc = _get_nc()
    in_maps = make_in_maps(idx, embed, encoder, encoder_v, decoder, lm_head)
    res = bass_utils.run_bass_kernel_spmd(nc, in_maps, core_ids=list(range(8)))
    out = np.empty((B, T, VOCAB), dtype=np.float32)
    for b in range(B):
        out[b] = res.results[4 * b]["logits_o"].reshape(T, VOCAB)
    return out


# revision 24
# speedup vs baseline: 1.0829x; 1.0066x over previous
"""Trainium2 Bass kernel for the BDH dense transformer (B=2, T=512, D=256, NH=4,
N=8192, 4 weight-tied layers, vocab 256).

Sharding: one (batch, head) pair per NeuronCore (2 x 4 = 8 cores). Per layer,
each core computes its head's yMLP partial (T, D); the 4 cores of a batch group
AllReduce the partials (with the replicated residual x/4 folded into the
payload) and every core redundantly applies layernorm so the activations stay
replicated within the group.

Collective overlap: phase D (y_latent/gate/yMLP) is split into two T-halves.
The first half's partial (t-chunks 0,1) is AllReduced while the second half
computes; the second collective is hidden under the NEXT layer's phase A first
T-half (which only needs the already-reduced t-chunks 0,1 of the new residual).

N-permutation: the score contraction over N is invariant under any permutation
of N applied consistently to (encoder cols, encoder_v cols, decoder rows, rope
freqs). We (1) de-interleave the rope pairs (even idx -> "lo", odd -> "hi") so
the pairwise rope becomes rotate-half form, then (2) interleave lo/hi CHUNK
QUADS: kernel chunks 8g..8g+7 = (lo 4g..4g+3, hi 4g..4g+3). A rope group is
then 8 consecutive chunks, relu/gate ops cover 2 adjacent chunks each, and
cos/sin tables are indexed by g.

Scores run in fp8e4 (DoubleRow, 2 N-chunks per matmul at 2x rate): the rope
scale C_ROPE is folded into the host cos/sin tables, qr tiles quantize to fp8
on the rope output, and the single 1/C^2 descale happens in the PSUM->smask
copy. All other matmuls are bf16 with fp32 PSUM accumulation.

All big DMAs are partition-major (host pre-transposes dec/cos/sin/emb/oneh/
lmh) so each of the 128 partition rows is one contiguous descriptor.
"""

import math

import numpy as np
import ml_dtypes

import concourse.bass as bass
import concourse.mybir as mybir
import concourse.tile as tile
from concourse import bacc
from concourse import bass_utils
from concourse.masks import make_identity

BF16 = ml_dtypes.bfloat16
F8E4 = ml_dtypes.float8_e4m3
F32 = mybir.dt.float32
BF = mybir.dt.bfloat16
F8 = mybir.dt.float8e4

# model dims (hardcoded per the problem spec)
B, T, D, NH, VOCAB = 2, 512, 256, 4, 256
N_LAYER = 4
MLP_MULT = 128
N = D * MLP_MULT // NH          # 8192 neurons per head
LN_EPS = 1e-5
TWO_PI = 2.0 * math.pi

NCH = N // 128                   # 64 partition-chunks of the neuron dim
HCH = NCH // 2                   # 32 chunks per rotate-half half
NGR = HCH // 4                   # 8 rope groups of 8 chunks (4 lo + 4 hi)
TC = T // 128                    # 4 t-chunks
DC = D // 128                    # 2 d-chunks
VC = VOCAB // 128                # 2 vocab-chunks
TH = T // 2                      # 256: columns per T-half

FP8_B = False                    # scores in fp8e4 DoubleRow (fp8 DVE writes too slow)
C_ENCV = 1024.0                  # fp8 scale on encoder_v (host-side)
C_YKV = 8.0                      # fp8 scale on the yKV cast (LN out can hit ~15-20)
C_ROPE = 32.0                    # rope-table scale folded into cos/sin
DESCALE = 1.0 / (C_ROPE * C_ROPE) if FP8_B else 1.0

RG = [[0, 1, 2, 3], [4, 5, 6, 7]]

AF = mybir.ActivationFunctionType
ALU = mybir.AluOpType
PM = mybir.MatmulPerfMode


def _layer_norm(nc, tmp, eps_tile, src, out_bf):
    """LN over the free dim (256 wide) of a (128, 256) tile -> bf16 out."""
    stats = tmp.tile([128, 6], F32, tag="bnst", bufs=2, name="ln_stats")
    nc.vector.bn_stats(stats, src)
    mv = tmp.tile([128, 2], F32, tag="bnmv", bufs=2, name="ln_mv")
    nc.vector.bn_aggr(mv, stats)
    std = tmp.tile([128, 1], F32, tag="std", bufs=2, name="ln_std")
    nc.scalar.activation(std, mv[:, 1:2], AF.Sqrt, bias=eps_tile)
    rstd = tmp.tile([128, 1], F32, tag="rstd", bufs=2, name="ln_rstd")
    nc.vector.reciprocal(rstd, std)
    nc.vector.tensor_scalar(
        out=out_bf, in0=src, scalar1=mv[:, 0:1], scalar2=rstd,
        op0=ALU.subtract, op1=ALU.mult,
    )


def _build_nc():
    nc = bacc.Bacc("TRN2", target_bir_lowering=False, debug=False, num_devices=8)

    # per-core external inputs (all partition-major)
    enc_w = nc.dram_tensor("enc_w", [DC, 128, N], BF, kind="ExternalInput")
    encv_w = nc.dram_tensor("encv_w", [DC, 128, N], BF, kind="ExternalInput")
    dec_w = nc.dram_tensor("dec_w", [128, NCH, D], BF, kind="ExternalInput")
    cos_w = nc.dram_tensor("cos_w", [128, NGR, 4, T], BF, kind="ExternalInput")
    sin_w = nc.dram_tensor("sin_w", [128, NGR, 4, T], BF, kind="ExternalInput")
    oneh_w = nc.dram_tensor("oneh_w", [128, VC, T], BF, kind="ExternalInput")
    emb_w = nc.dram_tensor("emb_w", [128, VC, D], BF, kind="ExternalInput")
    lmh_w = nc.dram_tensor("lmh_w", [128, DC, VOCAB], BF, kind="ExternalInput")
    logits_o = nc.dram_tensor("logits_o", [TC, 128, VOCAB], F32, kind="ExternalOutput")

    QRD = F8 if FP8_B else BF

    with tile.TileContext(nc) as tc:
        with (
            tc.tile_pool(name="wpool", bufs=1) as wp,
            tc.tile_pool(name="xspool", bufs=1) as xsp,
            tc.tile_pool(name="stream", bufs=3) as stream,
            tc.tile_pool(name="rope", bufs=2) as rope,
            tc.tile_pool(name="work", bufs=1) as work,
            tc.tile_pool(name="tmp", bufs=2) as tmp,
            tc.tile_pool(name="psA", bufs=2, space="PSUM") as psA,
            tc.tile_pool(name="psS", bufs=1, space="PSUM") as psS,
            tc.tile_pool(name="psT", bufs=1, space="PSUM") as psT,
            tc.tile_pool(name="dram", bufs=1, space="DRAM") as dram,
        ):
            # ---- resident weights (embedding inputs first so x0 can start
            # while the big encoder DMAs stream in) ----
            emb_sb = wp.tile([128, VC, D], BF, name="emb_sb")
            nc.sync.dma_start(emb_sb, emb_w.ap())
            oneh_sb = wp.tile([128, VC, T], BF, name="oneh_sb")
            nc.sync.dma_start(oneh_sb, oneh_w.ap())

            # warm up the collectives path with the same payload size as the
            # per-half AllReduce so layer 0 doesn't pay the first-call cost
            wcc_in = dram.tile([128, 2, D], BF, tag="wccin", name="wcc_in")
            wcc_out = dram.tile([128, 2, D], BF, tag="wccout", name="wcc_out")
            wcc_sb = tmp.tile([128, 2, D], BF, tag="wcc", bufs=1, name="wcc_sb")
            nc.vector.memset(wcc_sb, 0.0)
            nc.sync.dma_start(wcc_in, wcc_sb)
            nc.gpsimd.collective_compute(
                "AllReduce", ALU.add, replica_groups=RG,
                ins=[wcc_in.opt()], outs=[wcc_out.opt()],
            )

            ident = wp.tile([128, 128], BF, name="ident")
            make_identity(nc, ident)
            eps_tile = wp.tile([128, 1], F32, name="eps_tile")
            nc.vector.memset(eps_tile, LN_EPS)

            # encoder DMAs in slices (per-partition-contiguous columns)
            enc_sb = []
            for dc in range(DC):
                e = wp.tile([128, N], BF, tag=f"enc{dc}", name=f"enc_sb{dc}")
                enc_sb.append(e)
            NSL = 8
            sl = N // NSL
            for sli in range(NSL):
                for dc in range(DC):
                    nc.sync.dma_start(
                        enc_sb[dc][:, sli * sl:(sli + 1) * sl],
                        enc_w.ap()[dc, :, sli * sl:(sli + 1) * sl],
                    )
            encv_sb = []
            for dc in range(DC):
                ev = wp.tile([128, N], BF, tag=f"encv{dc}", name=f"encv_sb{dc}")
                encv_sb.append(ev)
            for sli in range(NSL):
                for dc in range(DC):
                    nc.sync.dma_start(
                        encv_sb[dc][:, sli * sl:(sli + 1) * sl],
                        encv_w.ap()[dc, :, sli * sl:(sli + 1) * sl],
                    )
            lmh_sb = wp.tile([128, DC, VOCAB], BF, name="lmh_sb")
            nc.sync.dma_start(lmh_sb, lmh_w.ap())

            # ---- persistent activations ----
            xs = xsp.tile([128, NCH, T], BF, name="xs")          # x_sparse / xy gate
            smask = work.tile([128, TC, T], BF, name="smask")    # masked scores (lhsT)
            x_bf = work.tile([128, TC, D], BF, name="x_bf")      # residual x (LN'd)
            xT = work.tile([128, DC, T], BF, name="xT")
            ykv_bf = work.tile([128, TC, D], BF, name="ykv_bf")
            ykvT = work.tile([128, 2, DC, TH], BF, name="ykvT")
            part_bf = work.tile([128, TC, D], BF, name="part_bf")

            # per-half collective buffers
            cc_in = [dram.tile([128, 2, D], BF, tag=f"ccin{h}", name=f"cc_in{h}")
                     for h in range(2)]
            cc_out = [dram.tile([128, 2, D], BF, tag=f"ccout{h}", name=f"cc_out{h}")
                      for h in range(2)]

            # zero the always-zero lower-left region of the masked score tiles
            for i in range(1, TC):
                nc.vector.memset(smask[:, i, 0:128 * i], 0.0)

            def emit_A_half(half):
                """x_latent columns [256h, 256h+256) for all chunks; relu in
                2-chunk pairs alternating scalar/vector. Chunk order follows
                the lo/hi pair interleave so rope group g is ready after
                chunks 4g..4g+3."""
                hs = slice(TH * half, TH * (half + 1))
                for pi in range(NCH // 2):
                    k0 = 2 * pi
                    # alternate between the psA lat buffers and the (idle
                    # during phase A) score banks so the relu round-trip
                    # never starves the tensor engine of PSUM space
                    if pi % 2 == 0:
                        lat = psA.tile([128, 2, TH], F32, tag="lat", bufs=3,
                                       name="lat_ps")
                    else:
                        lat = psS.tile([128, 2, TH], F32, tag=f"s{(pi // 2) % 4}",
                                       name="lat_ps")
                    for two in range(2):
                        k = k0 + two
                        for dc in range(DC):
                            nc.tensor.matmul(
                                lat[:, two, :],
                                lhsT=enc_sb[dc][:, 128 * k:128 * (k + 1)],
                                rhs=xT[:, dc, hs],
                                start=(dc == 0), stop=(dc == DC - 1),
                            )
                    # 2:1 scalar-biased relu split: keeps A tensor-paced
                    # while freeing vector time for the rope that paces B
                    if pi % 3 != 2:
                        nc.scalar.activation(xs[:, k0:k0 + 2, hs], lat, AF.Relu)
                    else:
                        nc.vector.tensor_scalar_max(xs[:, k0:k0 + 2, hs], lat, 0.0)

            def emit_tail(layer, half):
                """Consume the half's AllReduce result: LN -> x_bf, transpose
                into xT; on the last layer also the lm head + output DMA."""
                ag = tmp.tile([128, 2, D], BF, tag="ag", bufs=2, name="ag")
                nc.sync.dma_start(ag, cc_out[half])
                for jj in range(2):
                    j = 2 * half + jj
                    _layer_norm(nc, tmp, eps_tile, ag[:, jj, :], x_bf[:, j, :])
                    for dc in range(DC):
                        tp = psT.tile([128, 128], BF, tag="tr", name="tp_ps")
                        nc.tensor.transpose(
                            tp, x_bf[:, j, 128 * dc:128 * (dc + 1)], ident
                        )
                        nc.scalar.copy(xT[:, dc, 128 * j:128 * (j + 1)], tp)
                    if layer == N_LAYER - 1:
                        lg = psT.tile([128, VOCAB], F32, tag="tr", name="lg_ps")
                        for dc in range(DC):
                            nc.tensor.matmul(
                                lg, lhsT=xT[:, dc, 128 * j:128 * (j + 1)],
                                rhs=lmh_sb[:, dc, :],
                                start=(dc == 0), stop=(dc == DC - 1),
                            )
                        lgs = tmp.tile([128, VOCAB], F32, tag="lgs", name="lg_sb")
                        nc.scalar.copy(lgs, lg)
                        nc.sync.dma_start(logits_o.ap()[j], lgs)

            # ---- embedding: x0 = LN(onehot.T @ embed) ----
            for j in range(TC):
                x0 = psT.tile([128, D], F32, tag="tr", name="x0_ps")
                for vc in range(VC):
                    nc.tensor.matmul(
                        x0, lhsT=oneh_sb[:, vc, 128 * j:128 * (j + 1)],
                        rhs=emb_sb[:, vc, :],
                        start=(vc == 0), stop=(vc == VC - 1),
                    )
                _layer_norm(nc, tmp, eps_tile, x0, x_bf[:, j, :])
                for dc in range(DC):
                    tp = psT.tile([128, 128], BF, tag="tr", name="tp_ps")
                    nc.tensor.transpose(
                        tp, x_bf[:, j, 128 * dc:128 * (dc + 1)], ident
                    )
                    nc.scalar.copy(xT[:, dc, 128 * j:128 * (j + 1)], tp)
            emit_A_half(0)
            emit_A_half(1)

            # ---- layers ----
            for layer in range(N_LAYER):
                # phase B: rope (chunk groups of 4 = 2 lo-pairs + 2 hi-pairs)
                # + scores S = QR^T QR (upper-triangular blocks). With fp8,
                # each matmul is a DoubleRow pair (2 chunks, 2x rate).
                spsum = [
                    psS.tile([128, T], F32, tag=f"s{i}", name=f"score_ps{i}")
                    for i in range(TC)
                ]
                with nc.named_scope(f"L{layer}_B"):
                    for g in range(NGR):
                        lo = slice(8 * g, 8 * g + 4)
                        hi = slice(8 * g + 4, 8 * g + 8)
                        cosg = rope.tile([128, 4, T], BF, tag="cosg", name="cosg")
                        nc.sync.dma_start(cosg, cos_w.ap()[:, g])
                        sing = rope.tile([128, 4, T], BF, tag="sing", name="sing")
                        nc.sync.dma_start(sing, sin_w.ap()[:, g])
                        qrlo = rope.tile([128, 4, T], QRD, tag="qrlo", name="qrlo")
                        qrhi = rope.tile([128, 4, T], QRD, tag="qrhi", name="qrhi")
                        lo = slice(8 * g, 8 * g + 4)
                        hi = slice(8 * g + 4, 8 * g + 8)
                        ta = rope.tile([128, 4, T], BF, tag="ta", bufs=1, name="ropeta")
                        tb = rope.tile([128, 4, T], BF, tag="tb", bufs=1, name="ropetb")
                        nc.vector.tensor_mul(ta, xs[:, lo, :], cosg)
                        nc.vector.tensor_mul(tb, xs[:, hi, :], sing)
                        nc.vector.tensor_sub(qrlo, ta, tb)
                        ta2 = rope.tile([128, 4, T], BF, tag="ta", bufs=1, name="ropeta2")
                        tb2 = rope.tile([128, 4, T], BF, tag="tb", bufs=1, name="ropetb2")
                        nc.vector.tensor_mul(ta2, xs[:, hi, :], cosg)
                        nc.vector.tensor_mul(tb2, xs[:, lo, :], sing)
                        nc.vector.tensor_add(qrhi, ta2, tb2)
                        for qi, qr in enumerate((qrlo, qrhi)):
                            for kk in range(4):
                                first = (g == 0) and (qi == 0) and (kk == 0)
                                last = (g == NGR - 1) and (qi == 1) and (kk == 3)
                                for i in range(TC):
                                    nc.tensor.matmul(
                                        spsum[i][:, 128 * i:T],
                                        lhsT=qr[:, kk, 128 * i:128 * (i + 1)],
                                        rhs=qr[:, kk, 128 * i:T],
                                        start=first, stop=last,
                                    )

                # phase C: descale+mask scores, attention out, LN, transpose
                with nc.named_scope(f"L{layer}_C"):
                    for i in range(TC):
                        src = spsum[i][:, 128 * i:T]
                        dst = smask[:, i, 128 * i:T]
                        if i % 2 == 0:
                            nc.vector.tensor_scalar_mul(dst, src, DESCALE)
                        else:
                            nc.scalar.activation(dst, src, AF.Copy, scale=DESCALE)
                        diag = smask[:, i, 128 * i:128 * (i + 1)]
                        nc.gpsimd.affine_select(
                            out=diag, in_=diag, pattern=[[1, 128]], base=0,
                            channel_multiplier=-1, compare_op=ALU.is_gt, fill=0.0,
                        )
                        att = psS.tile([128, D], F32, tag=f"s{i}", name="att_ps")
                        for ii in range(i + 1):
                            nc.tensor.matmul(
                                att, lhsT=smask[:, ii, 128 * i:128 * (i + 1)],
                                rhs=x_bf[:, ii, :],
                                start=(ii == 0), stop=(ii == i),
                            )
                        _layer_norm(nc, tmp, eps_tile, att, ykv_bf[:, i, :])
                        for dc in range(DC):
                            tp = psT.tile([128, 128], BF, tag="tr", name="tp_ps")
                            nc.tensor.transpose(
                                tp, ykv_bf[:, i, 128 * dc:128 * (dc + 1)], ident
                            )
                            nc.scalar.copy(
                                ykvT[:, i // 2, dc, 128 * (i % 2):128 * (i % 2 + 1)],
                                tp,
                            )


                # phase D (per T-half): y_latent, fused relu+gate (2-chunk
                # pairs), yMLP partials, then the half's AllReduce with the
                # residual x/4 folded into the payload. Software-pipelined:
                # mlp matmuls lag one pair behind ylat/gate.
                for half in range(2):
                    hs = slice(TH * half, TH * (half + 1))
                    js = (2 * half, 2 * half + 1)
                    with nc.named_scope(f"L{layer}_D{half}"):
                        mlp = {
                            j: psS.tile([128, D], F32, tag=f"s{j}", name=f"mlp_ps{j}")
                            for j in js
                        }
                        NP = NCH // 2
                        MLAG = 3
                        dec_pairs = {}
                        for p in range(NP + MLAG):
                            if p < NP:
                                k0 = 2 * p
                                dp = stream.tile([128, 2, D], BF, tag=f"dec{half}",
                                                 bufs=6, name="dec_t")
                                nc.sync.dma_start(dp, dec_w.ap()[:, k0:k0 + 2, :])
                                dec_pairs[p] = dp
                                ylat = psA.tile([128, 2, TH], F32, tag="lat",
                                                bufs=3, name="ylat_ps")
                                for two in range(2):
                                    k = k0 + two
                                    for dc in range(DC):
                                        nc.tensor.matmul(
                                            ylat[:, two, :],
                                            lhsT=encv_sb[dc][:, 128 * k:128 * (k + 1)],
                                            rhs=ykvT[:, half, dc, :],
                                            start=(dc == 0), stop=(dc == DC - 1),
                                        )
                                # xy_sparse = relu(ylat) * x_sparse, fused
                                nc.vector.scalar_tensor_tensor(
                                    out=xs[:, k0:k0 + 2, hs], in0=ylat, scalar=0.0,
                                    in1=xs[:, k0:k0 + 2, hs],
                                    op0=ALU.max, op1=ALU.mult,
                                )
                            pm = p - MLAG
                            if pm >= 0:
                                dpm = dec_pairs.pop(pm)
                                for two in range(2):
                                    km = 2 * pm + two
                                    for j in js:
                                        nc.tensor.matmul(
                                            mlp[j],
                                            lhsT=xs[:, km, 128 * j:128 * (j + 1)],
                                            rhs=dpm[:, two, :],
                                            start=(km == 0), stop=(km == NCH - 1),
                                        )
                        # payload = yMLP partial + x/4 (residual folded in)
                        for j in js:
                            nc.vector.scalar_tensor_tensor(
                                out=part_bf[:, j, :], in0=x_bf[:, j, :],
                                scalar=0.25, in1=mlp[j],
                                op0=ALU.mult, op1=ALU.add,
                            )
                        nc.sync.dma_start(cc_in[half], part_bf[:, 2 * half:2 * half + 2, :])
                        nc.gpsimd.collective_compute(
                            "AllReduce", ALU.add, replica_groups=RG,
                            ins=[cc_in[half].opt()], outs=[cc_out[half].opt()],
                        )

                # tails + next layer's phase A halves, interleaved so the
                # second collective hides under A's first T-half
                with nc.named_scope(f"L{layer}_E0"):
                    emit_tail(layer, 0)
                if layer < N_LAYER - 1:
                    with nc.named_scope(f"L{layer + 1}_A0"):
                        emit_A_half(0)
                with nc.named_scope(f"L{layer}_E1"):
                    emit_tail(layer, 1)
                if layer < N_LAYER - 1:
                    with nc.named_scope(f"L{layer + 1}_A1"):
                        emit_A_half(1)

    nc.compile()
    return nc


_NC_CACHE = None


def _get_nc():
    global _NC_CACHE
    if _NC_CACHE is None:
        _NC_CACHE = _build_nc()
    return _NC_CACHE


def _host_tables():
    # de-interleave rope pairs (even first), then interleave lo/hi chunk
    # pairs: kernel chunk 4g+{0,1} = lo chunks 2g,2g+1 (= even source idx),
    # kernel chunk 4g+{2,3} = hi chunks 2g,2g+1 (= odd source idx)
    deint = np.concatenate([np.arange(0, N, 2), np.arange(1, N, 2)])
    chunk_order = []
    for g in range(NGR):
        chunk_order += [4 * g + i for i in range(4)]
        chunk_order += [HCH + 4 * g + i for i in range(4)]
    perm = deint.reshape(NCH, 128)[chunk_order].reshape(N)

    tq = np.floor(np.arange(N, dtype=np.float64) / 2.0) * 2.0
    freqs = 1.0 / (2.0 ** 16) ** (tq / N) / TWO_PI
    phases = np.arange(T)[None, :] * freqs[:, None]      # (N, T)
    p = (phases % 1.0) * TWO_PI
    scale = C_ROPE if FP8_B else 1.0
    cos_full = (np.cos(p) * scale)[perm]                 # (N, T), kernel order
    sin_full = (np.sin(p) * scale)[perm]
    cosg = np.empty((128, NGR, 4, T), dtype=BF16)
    sing = np.empty((128, NGR, 4, T), dtype=BF16)
    for g in range(NGR):
        for two in range(4):
            k = 8 * g + two                              # lo chunk of group
            cosg[:, g, two, :] = cos_full[128 * k:128 * (k + 1), :].astype(BF16)
            sing[:, g, two, :] = sin_full[128 * k:128 * (k + 1), :].astype(BF16)
    return perm, cosg, sing


def make_in_maps(idx, embed, encoder, encoder_v, decoder, lm_head):
    perm, cos_t, sin_t = _host_tables()
    idx = np.asarray(idx)
    embed = np.asarray(embed, dtype=np.float32)
    enc = np.asarray(encoder, dtype=np.float32)[:, :, perm].astype(BF16)
    encv = np.asarray(encoder_v, dtype=np.float32)[:, :, perm].astype(BF16)
    dec = np.asarray(decoder, dtype=np.float32).reshape(NH, N, D)[:, perm, :].astype(BF16)
    # decoder partition-major: [128, NCH, D]
    dec_pm = np.ascontiguousarray(
        dec.reshape(NH, NCH, 128, D).transpose(0, 2, 1, 3))
    emb_pm = np.ascontiguousarray(
        embed.astype(BF16).reshape(VC, 128, D).transpose(1, 0, 2))
    lmh_pm = np.ascontiguousarray(
        np.asarray(lm_head, dtype=np.float32).astype(BF16)
        .reshape(DC, 128, VOCAB).transpose(1, 0, 2))

    oneh = np.zeros((B, VOCAB, T), dtype=BF16)           # (b, v, t) = onehot^T
    for b in range(B):
        oneh[b, np.asarray(idx[b], dtype=np.int64), np.arange(T)] = 1
    oneh_pm = np.ascontiguousarray(
        oneh.reshape(B, VC, 128, T).transpose(0, 2, 1, 3))

    in_maps = []
    for c in range(8):
        b, h = c // 4, c % 4
        in_maps.append({
            "enc_w": np.ascontiguousarray(enc[h].reshape(DC, 128, N)),
            "encv_w": np.ascontiguousarray(encv[h].reshape(DC, 128, N)),
            "dec_w": dec_pm[h],
            "cos_w": cos_t,
            "sin_w": sin_t,
            "oneh_w": oneh_pm[b],
            "emb_w": emb_pm,
            "lmh_w": lmh_pm,
        })
    return in_maps


def kernel(idx, embed, encoder, encoder_v, decoder, lm_head):
    nc = _get_nc()
    in_maps = make_in_maps(idx, embed, encoder, encoder_v, decoder, lm_head)
    res = bass_utils.run_bass_kernel_spmd(nc, in_maps, core_ids=list(range(8)))
    out = np.empty((B, T, VOCAB), dtype=np.float32)
    for b in range(B):
        out[b] = res.results[4 * b]["logits_o"].reshape(T, VOCAB)
    return out
